# revision 38
# baseline (speedup 1.0000x reference)
"""Single transformer encoder layer on 8 Trainium2 NeuronCores.

Sharding: token-data-parallel, zero cross-core communication. Each core
owns 512 query tokens (4 cores per batch element) and computes K/V for
its whole batch locally, then attention, Wo, LN1, FFN, LN2.

Schedule (the point of this rewrite): QKV projections are computed
per-head-pair and software-pipelined with attention, so the ~147us of
softmax Exp on the Scalar engine hides under Tensor-engine matmuls
instead of gating them.  Score matmuls for the two heads of a pair are
emitted back-to-back on PE row-tiles (0,0)/(64,0) so they run
concurrently; scores for 8 key-chunks are batched between PE
mode-switches.  LayerNorm stats use [128,128] bf16 ones-matmuls (no
PE mode switch, 4x faster than fp32).  FFN relu/bias runs on the
Vector engine.  Weights (w1, wo) prefetch during attention.
"""

import sys

sys.path.insert(0, "/opt/trn_rl_repo")

import numpy as np
import ml_dtypes
from contextlib import ExitStack

import concourse.bass as bass
import concourse.mybir as mybir
import concourse.tile as tile
from concourse import bacc
import concourse.bass_utils as bass_utils

F32 = mybir.dt.float32
BF16 = mybir.dt.bfloat16
FP8 = mybir.dt.float8e4
AF = mybir.ActivationFunctionType
ALU = mybir.AluOpType

B, S, D = 2, 2048, 1024
H, DK, DV, DFF = 16, 64, 64, 4096
EPS = 1e-5
NCORES = 8
TOK = 512          # query tokens per core
SB = 2048          # batch tokens (K/V length)
NKC = SB // 128    # 16 key chunks
NDC = D // 128     # 8 feature chunks
NFC = DFF // 128   # 32 ffn chunks
NHP = H // 2       # 8 head pairs
NW1PRE = 8         # w1 fc-chunks prefetched during attention
SA = 2.0           # fp8 scale for softmax probs
SV = 32.0          # fp8 scale for V values
LOG_SA = float(np.log(SA))


def _bf(x):
    return np.ascontiguousarray(x.astype(ml_dtypes.bfloat16))


def _f32(x):
    return np.ascontiguousarray(x.astype(np.float32))


def _dram_chunked(t, ncols):
    """View a [R, ncols] DRAM tensor as [128, R//128, ncols]."""
    return t[:].rearrange("(c p) n -> p c n", p=128)


def build():
    nc = bacc.Bacc(name="encoder_layer", num_devices=NCORES)

    # ---- DRAM I/O ----
    xkT = nc.dram_tensor("xkT", [D, SB], BF16, kind="ExternalInput")
    xqT = nc.dram_tensor("xqT", [D, TOK], BF16, kind="ExternalInput")
    xqTf = nc.dram_tensor("xqTf", [D, TOK], F32, kind="ExternalInput")
    wqk = nc.dram_tensor("wqk", [D, 2 * D], BF16, kind="ExternalInput")
    wv = nc.dram_tensor("wv", [D, D], BF16, kind="ExternalInput")
    bqkv = nc.dram_tensor("bqkv", [3 * D, 1], F32, kind="ExternalInput")
    bv_row = nc.dram_tensor("bv_row", [1, D], BF16, kind="ExternalInput")
    wo = nc.dram_tensor("wo", [D, D], BF16, kind="ExternalInput")
    bo = nc.dram_tensor("bo", [D, 1], F32, kind="ExternalInput")
    w1 = nc.dram_tensor("w1", [D, DFF], BF16, kind="ExternalInput")
    b1 = nc.dram_tensor("b1", [DFF, 1], F32, kind="ExternalInput")
    w2 = nc.dram_tensor("w2", [DFF, D], BF16, kind="ExternalInput")
    b2 = nc.dram_tensor("b2", [D, 1], F32, kind="ExternalInput")
    g1 = nc.dram_tensor("g1", [D, 1], F32, kind="ExternalInput")
    be1 = nc.dram_tensor("be1", [D, 1], F32, kind="ExternalInput")
    g2 = nc.dram_tensor("g2", [D, 1], F32, kind="ExternalInput")
    be2 = nc.dram_tensor("be2", [D, 1], F32, kind="ExternalInput")
    outT = nc.dram_tensor("outT", [D, TOK], BF16, kind="ExternalOutput")

    with tile.TileContext(nc) as tc, ExitStack() as top:
        sp = top.enter_context(tc.tile_pool(name="smalls", bufs=1))

        ones_mm = sp.tile([128, 128], BF16)      # lhsT for column-sum matmuls
        nc.vector.memset(ones_mm, 1.0)
        ones_r = sp.tile([1, 128], F32)          # lhsT for partition-broadcast MMs
        nc.vector.memset(ones_r, 1.0)
        eps_t = sp.tile([1, 1], F32)
        nc.vector.memset(eps_t, EPS)
        lsa_t = sp.tile([128, 1], F32)
        nc.vector.memset(lsa_t, LOG_SA)
        bqkv_sb = sp.tile([128, 24], F32)
        nc.sync.dma_start(out=bqkv_sb,
                          in_=_dram_chunked(bqkv, 1).rearrange("p c n -> p (c n)"))

        # persistent activations (live across attention into the FFN)
        big = top.enter_context(tc.tile_pool(name="big", bufs=1))
        CT_sb = big.tile([128, NHP, TOK], BF16)        # ctx^T (heads on chunks)
        xres_sb = big.tile([128, NDC, TOK], F32)       # fp32 residual
        w1a_sb = big.tile([128, NDC, NW1PRE * 128], BF16)  # w1 fc 0..NW1PRE-1
        woa_sb = big.tile([128, NDC, TOK], F32)        # Wo output accumulator

        # attention-lifetime tensors (pool closed before the FFN phase)
        attn_cm = ExitStack()
        attx = attn_cm.enter_context(tc.tile_pool(name="attx", bufs=1))
        xk_sb = attx.tile([128, NDC, SB], BF16)        # x^T all batch tokens
        xq_sb = attx.tile([128, NDC, TOK], BF16)       # x^T own query tokens
        V_sb = attx.tile([128, NKC // 2, 2, H, DV + 1], FP8)  # V + ones col (x SV)

        # ---------------- QKV + attention, pipelined per head pair ----------------
        wqkp = attn_cm.enter_context(tc.tile_pool(name="wqkp", bufs=2))
        wot = attn_cm.enter_context(tc.tile_pool(name="wot", bufs=2))
        kqp = attn_cm.enter_context(tc.tile_pool(name="kqp", bufs=3))
        pqk = attn_cm.enter_context(tc.tile_pool(name="pqk", bufs=2, space="PSUM"))
        psS = attn_cm.enter_context(tc.tile_pool(name="psS", bufs=1, space="PSUM"))
        psC = attn_cm.enter_context(tc.tile_pool(name="psC", bufs=2, space="PSUM"))

        wvcm = ExitStack()
        wvp = wvcm.enter_context(tc.tile_pool(name="wvp", bufs=1))
        wv_sb = wvp.tile([128, NDC, D], BF16)

        # DMA order = issue order: everything that gates the first matmuls
        # goes first; K/V stream in by token-column quarters.
        wqk_t = [None] * NHP
        wqk_t[0] = wqkp.tile([128, NDC, 256], BF16, tag="wqk", name="wqk_t0")
        nc.scalar.dma_start(out=wqk_t[0],
                            in_=_dram_chunked(wqk, 2 * D)[:, :, 0:256])
        bv_bc = sp.tile([128, D], BF16)
        nc.scalar.dma_start(out=bv_bc, in_=bv_row[:].to_broadcast([128, D]))
        expwarm = sp.tile([1, 1], F32)
        nc.scalar.activation(out=expwarm, in_=eps_t, func=AF.Exp, scale=1.0)
        nc.gpsimd.dma_start(out=xq_sb, in_=_dram_chunked(xqT, TOK))
        nc.gpsimd.dma_start(out=wv_sb, in_=_dram_chunked(wv, D))
        for tq in range(4):
            eng = nc.sync if tq % 2 == 0 else nc.gpsimd
            eng.dma_start(out=xk_sb[:, :, tq * TOK:(tq + 1) * TOK],
                          in_=_dram_chunked(xkT, SB)[:, :, tq * TOK:(tq + 1) * TOK])
        nc.vector.memset(V_sb[:, :, :, :, DV:DV + 1], SV)
        nc.vector.memset(woa_sb, 0.0)

        def emit_qk(hp):
            """Q^T and K^T projections for head pair hp -> fresh kq tiles."""
            QT = kqp.tile([128, TOK], BF16, tag="qt")
            KT = kqp.tile([128, SB], BF16, tag="kt")
            w = wqk_t[hp]
            ps = pqk.tile([128, TOK], F32, tag="pqk")
            for dc in range(NDC):
                nc.tensor.matmul(ps, w[:, dc, 0:128], xq_sb[:, dc, :],
                                 start=(dc == 0), stop=(dc == NDC - 1))
            nc.vector.tensor_scalar(out=QT, in0=ps,
                                    scalar1=bqkv_sb[:, hp:hp + 1], scalar2=None,
                                    op0=ALU.add)
            for tt in range(SB // TOK):
                ps = pqk.tile([128, TOK], F32, tag="pqk")
                for dc in range(NDC):
                    nc.tensor.matmul(ps, w[:, dc, 128:256],
                                     xk_sb[:, dc, tt * TOK:(tt + 1) * TOK],
                                     start=(dc == 0), stop=(dc == NDC - 1))
                nc.vector.tensor_scalar(out=KT[:, tt * TOK:(tt + 1) * TOK],
                                        in0=ps, scalar1=bqkv_sb[:, 8 + hp:9 + hp],
                                        scalar2=None, op0=ALU.add)
            return QT, KT

        qkt = [None] * NHP
        qkt[0] = emit_qk(0)

        # V projections for all heads (fills PE while first exps run)
        for tc_ in range(NKC):
            ps0 = pqk.tile([128, TOK], F32, tag="pqk")
            ps1 = pqk.tile([128, TOK], F32, tag="pqk")
            for dc in range(NDC):
                lhs = xk_sb[:, dc, tc_ * 128:(tc_ + 1) * 128]
                nc.tensor.matmul(ps0, lhs, wv_sb[:, dc, 0:512],
                                 start=(dc == 0), stop=(dc == NDC - 1))
                nc.tensor.matmul(ps1, lhs, wv_sb[:, dc, 512:1024],
                                 start=(dc == 0), stop=(dc == NDC - 1))
            nc.vector.scalar_tensor_tensor(
                out=V_sb[:, tc_ // 2, tc_ % 2, 0:8, 0:DV],
                in0=ps0[:].rearrange("p (h j) -> p h j", j=DV),
                scalar=SV, in1=bv_bc[:, 0:512].rearrange("p (h j) -> p h j", j=DV),
                op0=ALU.mult, op1=ALU.add)
            nc.vector.scalar_tensor_tensor(
                out=V_sb[:, tc_ // 2, tc_ % 2, 8:16, 0:DV],
                in0=ps1[:].rearrange("p (h j) -> p h j", j=DV),
                scalar=SV, in1=bv_bc[:, 512:1024].rearrange("p (h j) -> p h j", j=DV),
                op0=ALU.mult, op1=ALU.add)

        wvcm.close()  # wv region reusable by pools opened below
        ap = attn_cm.enter_context(tc.tile_pool(name="apool", bufs=2))
        npool = attn_cm.enter_context(tc.tile_pool(name="npool", bufs=1))

        for hp in range(NHP):
            # prefetch next head pair's projection weights + bulk weights
            if hp + 1 < NHP:
                wqk_t[hp + 1] = wqkp.tile([128, NDC, 256], BF16, tag="wqk",
                                         name=f"wqk_t{hp + 1}")
                nc.sync.dma_start(
                    out=wqk_t[hp + 1],
                    in_=_dram_chunked(wqk, 2 * D)[:, :, (hp + 1) * 256:(hp + 2) * 256])
            wo_t = wot.tile([128, D], BF16, tag="wo", name=f"wo_t{hp}")
            nc.sync.dma_start(out=wo_t, in_=_dram_chunked(wo, D)[:, hp, :])
            if hp < 4:  # w1 chunks 0..7: 0.5MB per hp at hp=0..3
                csl = slice(hp * 2 * 128, (hp + 1) * 2 * 128)
                nc.sync.dma_start(out=w1a_sb[:, :, csl],
                                  in_=_dram_chunked(w1, DFF)[:, :, csl])
            else:  # fp32 residual at hp=4..7
                qsl = slice((hp - 4) * 2, (hp - 3) * 2)
                nc.sync.dma_start(out=xres_sb[:, qsl, :],
                                  in_=_dram_chunked(xqTf, TOK)[:, qsl, :])

            QT, KT = qkt[hp]
            ctx0 = psC.tile([DV + 1, TOK], F32, tag="ctx")
            ctx1 = psC.tile([DV + 1, TOK], F32, tag="ctx")
            for half in range(2):
                a2 = ap.tile([128, NKC // 4, 2, 2 * TOK], FP8, tag="a2")
                # scores for 8 key chunks, both heads, batched (one PE mode);
                # one 4-bank PSUM tile = one Exp ACTIVATE per 2 key chunks
                for kcp in range(NKC // 4):
                    s2 = psS.tile([128, 4 * TOK], F32, tag="s")
                    for j2 in range(2):
                        kcg = half * (NKC // 2) + kcp * 2 + j2
                        ksl = slice(kcg * 128, (kcg + 1) * 128)
                        osl0 = slice(j2 * 2 * TOK, j2 * 2 * TOK + TOK)
                        osl1 = slice(j2 * 2 * TOK + TOK, (j2 + 1) * 2 * TOK)
                        nc.tensor.matmul(s2[:, osl0], KT[0:64, ksl], QT[0:64, :],
                                         start=True, stop=True)
                        nc.tensor.matmul(s2[:, osl1], KT[64:128, ksl],
                                         QT[64:128, :], start=True, stop=True)
                    nc.scalar.activation(out=a2[:, kcp, :, :], in_=s2,
                                         func=AF.Exp, scale=1.0 / np.sqrt(DK),
                                         bias=lsa_t[:, 0:1])
                # ctx accumulation: fp8 DoubleRow over key pairs
                for kcp in range(NKC // 4):
                    kpg = half * (NKC // 4) + kcp
                    nc.tensor.matmul(ctx0, V_sb[:, kpg, :, 2 * hp, :],
                                     a2[:, kcp, :, 0:TOK],
                                     perf_mode=mybir.MatmulPerfMode.DoubleRow,
                                     start=(kpg == 0), stop=(kpg == NKC // 2 - 1))
                    nc.tensor.matmul(ctx1, V_sb[:, kpg, :, 2 * hp + 1, :],
                                     a2[:, kcp, :, TOK:2 * TOK],
                                     perf_mode=mybir.MatmulPerfMode.DoubleRow,
                                     start=(kpg == 0), stop=(kpg == NKC // 2 - 1))
            if hp + 1 < NHP:
                qkt[hp + 1] = emit_qk(hp + 1)
            # softmax normalization: divide by denominator (ones row of V)
            for j, ctx_ps in ((0, ctx0), (1, ctx1)):
                off = j * 64
                d_t = npool.tile([1, TOK], F32, tag="d")
                nc.vector.tensor_copy(out=d_t, in_=ctx_ps[DV:DV + 1, :])
                r_t = npool.tile([1, TOK], F32, tag="r")
                nc.vector.reciprocal_approx_fast(out=r_t, in_=d_t)
                rb_t = npool.tile([64, TOK], F32, tag="rb")
                nc.gpsimd.partition_broadcast(rb_t[:], r_t[:], channels=64)
                nc.vector.tensor_tensor(out=CT_sb[off:off + 64, hp, :],
                                        in0=ctx_ps[0:DV, :], in1=rb_t, op=ALU.mult)
            # Wo contribution of this head pair, accumulated in SBUF (GpSimd)
            for mc in range(NDC):
                psw = pqk.tile([128, TOK], F32, tag="pqk", name=f"psw{hp}_{mc}")
                nc.tensor.matmul(psw, wo_t[:, mc * 128:(mc + 1) * 128],
                                 CT_sb[:, hp, :], start=True, stop=True)
                nc.vector.tensor_tensor(out=woa_sb[:, mc, :], in0=woa_sb[:, mc, :],
                                        in1=psw, op=ALU.add)

        sqwarm = sp.tile([1, 1], F32)
        nc.scalar.activation(out=sqwarm, in_=eps_t, func=AF.Sqrt, bias=eps_t,
                             scale=1.0)

        attn_cm.close()  # free attention pools (xk, xq, V, wv, a2, KT/QT)

        bo_sb = sp.tile([128, 8], F32)
        nc.sync.dma_start(out=bo_sb, in_=_dram_chunked(bo, 1).rearrange("p c n -> p (c n)"))
        b1_sb = sp.tile([128, 32], F32)
        nc.sync.dma_start(out=b1_sb, in_=_dram_chunked(b1, 1).rearrange("p c n -> p (c n)"))
        b2_sb = sp.tile([128, 8], F32)
        nc.sync.dma_start(out=b2_sb, in_=_dram_chunked(b2, 1).rearrange("p c n -> p (c n)"))
        g1_sb = sp.tile([128, 8], F32)
        nc.sync.dma_start(out=g1_sb, in_=_dram_chunked(g1, 1).rearrange("p c n -> p (c n)"))
        be1_sb = sp.tile([128, 8], F32)
        nc.sync.dma_start(out=be1_sb, in_=_dram_chunked(be1, 1).rearrange("p c n -> p (c n)"))
        g2_sb = sp.tile([128, 8], F32)
        nc.sync.dma_start(out=g2_sb, in_=_dram_chunked(g2, 1).rearrange("p c n -> p (c n)"))
        be2_sb = sp.tile([128, 8], F32)
        nc.sync.dma_start(out=be2_sb, in_=_dram_chunked(be2, 1).rearrange("p c n -> p (c n)"))

        # ---------------- residual + LN1 (Wo already accumulated) ----------
        with ExitStack() as ph3:
            lnp = ph3.enter_context(tc.tile_pool(name="lnp", bufs=2))
            hp3 = ph3.enter_context(tc.tile_pool(name="hpool3", bufs=1))

            hT_sb = hp3.tile([128, NFC, TOK], BF16)
            ln1b_sb = hp3.tile([128, NDC, TOK], BF16)      # LN1 out (FFN rhs + residual)
            y1_sb = woa_sb

            w2p = ph3.enter_context(tc.tile_pool(name="w2p", bufs=3))
            psA = ExitStack()
            w1s = psA.enter_context(tc.tile_pool(name="w1s", bufs=3))
            psSt = psA.enter_context(tc.tile_pool(name="psSt", bufs=1, space="PSUM"))
            psbc = psA.enter_context(tc.tile_pool(name="psbc", bufs=1, space="PSUM"))
            psF = psA.enter_context(tc.tile_pool(name="psF", bufs=4, space="PSUM"))

            s_ps = psSt.tile([128, TOK], F32, tag="sum")
            q_ps = psSt.tile([128, TOK], F32, tag="sq")
            for mc in range(NDC):
                nc.vector.scalar_tensor_tensor(out=y1_sb[:, mc, :],
                                               in0=woa_sb[:, mc, :],
                                               scalar=bo_sb[:, mc:mc + 1],
                                               in1=xres_sb[:, mc, :],
                                               op0=ALU.add, op1=ALU.add)
                yb_t = lnp.tile([128, TOK], BF16, tag="yb")
                nc.vector.tensor_copy(out=yb_t, in_=y1_sb[:, mc, :])
                sq_t = lnp.tile([128, TOK], BF16, tag="sq")
                nc.scalar.square(out=sq_t, in_=y1_sb[:, mc, :])
                nc.tensor.matmul(s_ps, ones_mm, yb_t,
                                 start=(mc == 0), stop=(mc == NDC - 1))
                nc.tensor.matmul(q_ps, ones_mm, sq_t,
                                 start=(mc == 0), stop=(mc == NDC - 1))

            _ln_norm(nc, lnp, s_ps, q_ps, eps_t, y1_sb, g1_sb, be1_sb,
                     None, ln1b_sb, None, ones_r=ones_r, psbc=psbc)

            # FFN1: groups of 4 fc chunks; dc-ordered accumulation pipelines
            # with LN1 chunk production above
            w2_tiles = {}
            for mc in range(2):
                w2_tiles[mc] = w2p.tile([128, NFC, 128], BF16, tag="w2",
                                        name=f"w2_t{mc}")
                nc.sync.dma_start(out=w2_tiles[mc],
                                  in_=_dram_chunked(w2, D)[:, :, mc * 128:(mc + 1) * 128])
            for g in range(NFC // 4):
                pss = [psF.tile([128, TOK], F32, tag="f", name=f"psf{g}_{i}")
                       for i in range(4)]
                if g * 4 >= NW1PRE:
                    wt = w1s.tile([128, NDC, 4 * 128], BF16, tag="w1s")
                    nc.sync.dma_start(
                        out=wt,
                        in_=_dram_chunked(w1, DFF)[:, :, g * 512:(g + 1) * 512])
                for dc in range(NDC):
                    for f in range(4):
                        fc = g * 4 + f
                        if fc < NW1PRE:
                            lhs = w1a_sb[:, dc, fc * 128:(fc + 1) * 128]
                        else:
                            lhs = wt[:, dc, f * 128:(f + 1) * 128]
                        nc.tensor.matmul(pss[f], lhs, ln1b_sb[:, dc, :],
                                         start=(dc == 0), stop=(dc == NDC - 1))
                for f in range(4):
                    fc = g * 4 + f
                    nc.vector.tensor_scalar(out=hT_sb[:, fc, :], in0=pss[f],
                                            scalar1=b1_sb[:, fc:fc + 1],
                                            scalar2=0.0, op0=ALU.add, op1=ALU.max)

            psA.close()  # free Wo/FFN1 PSUM pools

            # ---------------- FFN2 + residual + LN2 ----------------
            psF2 = ph3.enter_context(tc.tile_pool(name="psF2", bufs=3, space="PSUM"))
            psSt2 = ph3.enter_context(tc.tile_pool(name="psSt2", bufs=1, space="PSUM"))
            psbc2 = ph3.enter_context(tc.tile_pool(name="psbc2", bufs=1, space="PSUM"))
            y2_sb = hp3.tile([128, NDC, TOK], F32)

            s2_ps = psSt2.tile([128, TOK], F32, tag="sum2")
            q2_ps = psSt2.tile([128, TOK], F32, tag="sq2")
            for mc in range(NDC):
                if mc in w2_tiles:
                    w2_t = w2_tiles[mc]
                else:
                    w2_t = w2p.tile([128, NFC, 128], BF16, tag="w2",
                                    name=f"w2_t{mc}")
                    nc.sync.dma_start(
                        out=w2_t,
                        in_=_dram_chunked(w2, D)[:, :, mc * 128:(mc + 1) * 128])
                ps = psF2.tile([128, TOK], F32, tag="f2")
                for fc in range(NFC):
                    nc.tensor.matmul(ps, w2_t[:, fc, :], hT_sb[:, fc, :],
                                     start=(fc == 0), stop=(fc == NFC - 1))
                nc.vector.scalar_tensor_tensor(out=y2_sb[:, mc, :], in0=ps,
                                               scalar=b2_sb[:, mc:mc + 1],
                                               in1=ln1b_sb[:, mc, :],
                                               op0=ALU.add, op1=ALU.add)
                yb_t = lnp.tile([128, TOK], BF16, tag="yb")
                nc.vector.tensor_copy(out=yb_t, in_=y2_sb[:, mc, :])
                sq_t = lnp.tile([128, TOK], BF16, tag="sq")
                nc.scalar.square(out=sq_t, in_=y2_sb[:, mc, :])
                nc.tensor.matmul(s2_ps, ones_mm, yb_t,
                                 start=(mc == 0), stop=(mc == NDC - 1))
                nc.tensor.matmul(q2_ps, ones_mm, sq_t,
                                 start=(mc == 0), stop=(mc == NDC - 1))

            _ln_norm(nc, lnp, s2_ps, q2_ps, eps_t, y2_sb, g2_sb, be2_sb,
                     None, None, (_dram_chunked(outT, TOK), nc),
                     ones_r=ones_r, psbc=psbc2)

    nc.compile()
    return nc


def _ln_norm(nc, lnp, s_ps, q_ps, eps_t, y_sb, g_sb, be_sb,
             out_f32, out_bf16, out_dma=None, ones_r=None, psbc=None):
    """Finish LayerNorm given accumulated sum/sumsq PSUM tiles (row 0)."""
    mean_t = lnp.tile([1, TOK], F32, tag="mean")
    nc.scalar.mul(out=mean_t, in_=s_ps[0:1, :], mul=1.0 / D)
    msq_t = lnp.tile([1, TOK], F32, tag="msq")
    nc.scalar.mul(out=msq_t, in_=q_ps[0:1, :], mul=1.0 / D)
    var_t = lnp.tile([1, TOK], F32, tag="var")
    nc.vector.tensor_tensor(out=var_t, in0=mean_t, in1=mean_t, op=ALU.mult)
    nc.vector.tensor_sub(out=var_t, in0=msq_t, in1=var_t)
    std_t = lnp.tile([1, TOK], F32, tag="std")
    nc.scalar.activation(out=std_t, in_=var_t, func=AF.Sqrt, bias=eps_t, scale=1.0)
    rstd_t = lnp.tile([1, TOK], F32, tag="rstd")
    nc.vector.reciprocal_approx_fast(out=rstd_t, in_=std_t)
    # broadcast mean/rstd across partitions with [1]-contraction matmuls
    # (Tensor engine is idle here; avoids GpSimd library thrash)
    mb_ps = psbc.tile([128, TOK], F32, tag="mb")
    nc.tensor.matmul(mb_ps, ones_r, mean_t, start=True, stop=True)
    rb_ps = psbc.tile([128, TOK], F32, tag="rb")
    nc.tensor.matmul(rb_ps, ones_r, rstd_t, start=True, stop=True)
    for mc in range(NDC):
        t1 = lnp.tile([128, TOK], BF16, tag="t1")
        nc.vector.tensor_sub(out=t1, in0=y_sb[:, mc, :], in1=mb_ps)
        t2 = lnp.tile([128, TOK], BF16, tag="t2")
        nc.vector.scalar_tensor_tensor(out=t2, in0=t1, scalar=g_sb[:, mc:mc + 1],
                                       in1=rb_ps, op0=ALU.mult, op1=ALU.mult)
        if out_f32 is not None or out_dma is not None:
            if out_f32 is not None:
                o_t = out_f32[:, mc, :]
            else:
                o_t = lnp.tile([128, TOK], BF16, tag="o3")
            nc.vector.tensor_scalar(out=o_t, in0=t2,
                                    scalar1=be_sb[:, mc:mc + 1], scalar2=None,
                                    op0=ALU.add)
            if out_dma is not None:
                dram, _nc = out_dma
                _nc.sync.dma_start(out=dram[:, mc, :], in_=o_t)
        if out_bf16 is not None:
            nc.vector.tensor_scalar(out=out_bf16[:, mc, :], in0=t2,
                                    scalar1=be_sb[:, mc:mc + 1], scalar2=None,
                                    op0=ALU.add)



_COMPILED = None
_LAST_IN_MAPS = None


def kernel(**inputs):
    global _COMPILED, _LAST_IN_MAPS
    ins = {k: np.asarray(v) for k, v in inputs.items()}
    x = _f32(ins["x"])
    Wq, bq = ins["Wq"], ins["bq"]
    Wk, bk = ins["Wk"], ins["bk"]
    Wv, bv = ins["Wv"], ins["bv"]
    Wo, bo = ins["Wo"], ins["bo"]
    W1, b1 = ins["W1"], ins["b1"]
    W2, b2 = ins["W2"], ins["b2"]
    g1, be1 = ins["g1"], ins["be1"]
    g2, be2 = ins["g2"], ins["be2"]

    wq2 = Wq.transpose(1, 0, 2).reshape(D, H * DK)
    wk2 = Wk.transpose(1, 0, 2).reshape(D, H * DK)
    # per-head-pair interleave: [q_hp (128) | k_hp (128)] blocks
    wqk = np.empty((D, 2 * D), np.float32)
    for hp in range(NHP):
        wqk[:, hp * 256:hp * 256 + 128] = wq2[:, hp * 128:(hp + 1) * 128]
        wqk[:, hp * 256 + 128:(hp + 1) * 256] = wk2[:, hp * 128:(hp + 1) * 128]
    bqkv = np.concatenate([bq.reshape(-1), bk.reshape(-1), bv.reshape(-1)])

    shared = {
        "wqk": _bf(wqk),
        "wv": _bf(Wv.transpose(1, 0, 2).reshape(D, H * DV)),
        "bqkv": _f32(bqkv.reshape(3 * D, 1)),
        "bv_row": _bf(bv.reshape(1, H * DV) * SV),
        "wo": _bf(Wo),
        "bo": _f32(bo.reshape(D, 1)),
        "w1": _bf(W1),
        "b1": _f32(b1.reshape(DFF, 1)),
        "w2": _bf(W2),
        "b2": _f32(b2.reshape(D, 1)),
        "g1": _f32(g1.reshape(D, 1)),
        "be1": _f32(be1.reshape(D, 1)),
        "g2": _f32(g2.reshape(D, 1)),
        "be2": _f32(be2.reshape(D, 1)),
    }

    in_maps = []
    for c in range(NCORES):
        b, qoff = c // 4, (c % 4) * TOK
        xb = x[b]                        # (S, D) fp32
        xkT = np.ascontiguousarray(xb.T)         # (D, S)
        xqT = np.ascontiguousarray(xb[qoff:qoff + TOK].T)  # (D, TOK)
        m = dict(shared)
        m["xkT"] = _bf(xkT)
        m["xqT"] = _bf(xqT)
        m["xqTf"] = _f32(xqT)
        in_maps.append(m)
    _LAST_IN_MAPS = in_maps

    if _COMPILED is None:
        _COMPILED = build()
    res = bass_utils.run_bass_kernel_spmd(_COMPILED, in_maps,
                                          core_ids=list(range(NCORES)))
    out = np.empty((B, S, D), np.float32)
    for c in range(NCORES):
        b, qoff = c // 4, (c % 4) * TOK
        out[b, qoff:qoff + TOK, :] = res.results[c]["outT"].T.astype(np.float32)
    return out


# revision 39
# speedup vs baseline: 1.0541x; 1.0541x over previous
"""Single transformer encoder layer on 8 Trainium2 NeuronCores.

Sharding: token-data-parallel, zero cross-core communication. Each core
owns 512 query tokens (4 cores per batch element) and computes K/V for
its whole batch locally, then attention, Wo, LN1, FFN, LN2.

Schedule (the point of this rewrite): QKV projections are computed
per-head-pair and software-pipelined with attention, so the ~147us of
softmax Exp on the Scalar engine hides under Tensor-engine matmuls
instead of gating them.  Score matmuls for the two heads of a pair are
emitted back-to-back on PE row-tiles (0,0)/(64,0) so they run
concurrently; scores for 8 key-chunks are batched between PE
mode-switches.  LayerNorm stats use [128,128] bf16 ones-matmuls (no
PE mode switch, 4x faster than fp32).  FFN relu/bias runs on the
Vector engine.  Weights (w1, wo) prefetch during attention.
"""

import sys

sys.path.insert(0, "/opt/trn_rl_repo")

import numpy as np
import ml_dtypes
from contextlib import ExitStack

import concourse.bass as bass
import concourse.mybir as mybir
import concourse.tile as tile
from concourse import bacc
import concourse.bass_utils as bass_utils

F32 = mybir.dt.float32
BF16 = mybir.dt.bfloat16
FP8 = mybir.dt.float8e4
AF = mybir.ActivationFunctionType
ALU = mybir.AluOpType

B, S, D = 2, 2048, 1024
H, DK, DV, DFF = 16, 64, 64, 4096
EPS = 1e-5
NCORES = 8
TOK = 512          # query tokens per core
SB = 2048          # batch tokens (K/V length)
NKC = SB // 128    # 16 key chunks
NDC = D // 128     # 8 feature chunks
NFC = DFF // 128   # 32 ffn chunks
NHP = H // 2       # 8 head pairs
NW1PRE = 8         # w1 fc-chunks prefetched during attention
SA = 2.0           # fp8 scale for softmax probs
SV = 32.0          # fp8 scale for V values
LOG_SA = float(np.log(SA))


def _bf(x):
    return np.ascontiguousarray(x.astype(ml_dtypes.bfloat16))


def _f32(x):
    return np.ascontiguousarray(x.astype(np.float32))


def _dram_chunked(t, ncols):
    """View a [R, ncols] DRAM tensor as [128, R//128, ncols]."""
    return t[:].rearrange("(c p) n -> p c n", p=128)


def build():
    nc = bacc.Bacc(name="encoder_layer", num_devices=NCORES)

    # ---- DRAM I/O ----
    xkT = nc.dram_tensor("xkT", [D, SB], BF16, kind="ExternalInput")
    xqT = nc.dram_tensor("xqT", [D, TOK], BF16, kind="ExternalInput")
    xqTf = nc.dram_tensor("xqTf", [D, TOK], F32, kind="ExternalInput")
    wqk = nc.dram_tensor("wqk", [D, 2 * D], BF16, kind="ExternalInput")
    wv = nc.dram_tensor("wv", [D, D], BF16, kind="ExternalInput")
    bqkv = nc.dram_tensor("bqkv", [3 * D, 1], F32, kind="ExternalInput")
    bv_row = nc.dram_tensor("bv_row", [1, D], BF16, kind="ExternalInput")
    wo = nc.dram_tensor("wo", [D, D], BF16, kind="ExternalInput")
    bo = nc.dram_tensor("bo", [D, 1], F32, kind="ExternalInput")
    w1 = nc.dram_tensor("w1", [D, DFF], BF16, kind="ExternalInput")
    b1 = nc.dram_tensor("b1", [DFF, 1], F32, kind="ExternalInput")
    w2 = nc.dram_tensor("w2", [DFF, D], BF16, kind="ExternalInput")
    b2 = nc.dram_tensor("b2", [D, 1], F32, kind="ExternalInput")
    g1 = nc.dram_tensor("g1", [D, 1], F32, kind="ExternalInput")
    be1 = nc.dram_tensor("be1", [D, 1], F32, kind="ExternalInput")
    g2 = nc.dram_tensor("g2", [D, 1], F32, kind="ExternalInput")
    be2 = nc.dram_tensor("be2", [D, 1], F32, kind="ExternalInput")
    outT = nc.dram_tensor("outT", [D, TOK], BF16, kind="ExternalOutput")

    with tile.TileContext(nc) as tc, ExitStack() as top:
        sp = top.enter_context(tc.tile_pool(name="smalls", bufs=1))

        ones_mm = sp.tile([128, 128], BF16)      # lhsT for column-sum matmuls
        nc.vector.memset(ones_mm, 1.0)
        ones_r = sp.tile([1, 128], F32)          # lhsT for partition-broadcast MMs
        nc.vector.memset(ones_r, 1.0)
        eps_t = sp.tile([1, 1], F32)
        nc.vector.memset(eps_t, EPS)
        lsa_t = sp.tile([128, 1], F32)
        nc.vector.memset(lsa_t, LOG_SA)
        bqkv_sb = sp.tile([128, 24], F32)
        nc.sync.dma_start(out=bqkv_sb,
                          in_=_dram_chunked(bqkv, 1).rearrange("p c n -> p (c n)"))

        # persistent activations (live across attention into the FFN)
        big = top.enter_context(tc.tile_pool(name="big", bufs=1))
        CT_sb = big.tile([128, NHP, TOK], BF16)        # ctx^T (heads on chunks)
        xres_sb = big.tile([128, NDC, TOK], F32)       # fp32 residual
        w1a_sb = big.tile([128, NDC, NW1PRE * 128], BF16)  # w1 fc 0..NW1PRE-1
        woa_sb = big.tile([128, NDC, TOK], F32)        # Wo output accumulator

        # attention-lifetime tensors (pool closed before the FFN phase)
        attn_cm = ExitStack()
        attx = attn_cm.enter_context(tc.tile_pool(name="attx", bufs=1))
        xk_sb = attx.tile([128, NDC, SB], BF16)        # x^T all batch tokens
        xq_sb = attx.tile([128, NDC, TOK], BF16)       # x^T own query tokens
        V_sb = attx.tile([128, NKC // 2, 2, H, DV + 1], FP8)  # V + ones col (x SV)

        # ---------------- QKV + attention, pipelined per head pair ----------------
        wqkp = attn_cm.enter_context(tc.tile_pool(name="wqkp", bufs=2))
        wot = attn_cm.enter_context(tc.tile_pool(name="wot", bufs=2))
        kqp = attn_cm.enter_context(tc.tile_pool(name="kqp", bufs=3))
        pqk = attn_cm.enter_context(tc.tile_pool(name="pqk", bufs=2, space="PSUM"))
        psS = attn_cm.enter_context(tc.tile_pool(name="psS", bufs=2, space="PSUM"))
        psC = attn_cm.enter_context(tc.tile_pool(name="psC", bufs=2, space="PSUM"))

        wvcm = ExitStack()
        wvp = wvcm.enter_context(tc.tile_pool(name="wvp", bufs=1))
        wv_sb = wvp.tile([128, NDC, D], BF16)

        # DMA order = issue order: everything that gates the first matmuls
        # goes first; K/V stream in by token-column quarters.
        wqk_t = [None] * NHP
        wqk_t[0] = wqkp.tile([128, NDC, 256], BF16, tag="wqk", name="wqk_t0")
        nc.scalar.dma_start(out=wqk_t[0],
                            in_=_dram_chunked(wqk, 2 * D)[:, :, 0:256])
        bv_bc = sp.tile([128, D], BF16)
        nc.scalar.dma_start(out=bv_bc, in_=bv_row[:].to_broadcast([128, D]))
        expwarm = sp.tile([1, 1], F32)
        nc.scalar.activation(out=expwarm, in_=eps_t, func=AF.Exp, scale=1.0)
        nc.gpsimd.dma_start(out=xq_sb, in_=_dram_chunked(xqT, TOK))
        nc.gpsimd.dma_start(out=wv_sb, in_=_dram_chunked(wv, D))
        for tq in range(4):
            eng = nc.sync if tq % 2 == 0 else nc.gpsimd
            eng.dma_start(out=xk_sb[:, :, tq * TOK:(tq + 1) * TOK],
                          in_=_dram_chunked(xkT, SB)[:, :, tq * TOK:(tq + 1) * TOK])
        nc.vector.memset(V_sb[:, :, :, :, DV:DV + 1], SV)
        nc.vector.memset(woa_sb, 0.0)

        def emit_qk(hp):
            """Q^T and K^T projections for head pair hp -> fresh kq tiles."""
            QT = kqp.tile([128, TOK], BF16, tag="qt")
            KT = kqp.tile([128, SB], BF16, tag="kt")
            w = wqk_t[hp]
            ps = pqk.tile([128, TOK], F32, tag="pqk")
            for dc in range(NDC):
                nc.tensor.matmul(ps, w[:, dc, 0:128], xq_sb[:, dc, :],
                                 start=(dc == 0), stop=(dc == NDC - 1))
            nc.vector.tensor_scalar(out=QT, in0=ps,
                                    scalar1=bqkv_sb[:, hp:hp + 1], scalar2=None,
                                    op0=ALU.add)
            for tt in range(SB // TOK):
                ps = pqk.tile([128, TOK], F32, tag="pqk")
                for dc in range(NDC):
                    nc.tensor.matmul(ps, w[:, dc, 128:256],
                                     xk_sb[:, dc, tt * TOK:(tt + 1) * TOK],
                                     start=(dc == 0), stop=(dc == NDC - 1))
                nc.vector.tensor_scalar(out=KT[:, tt * TOK:(tt + 1) * TOK],
                                        in0=ps, scalar1=bqkv_sb[:, 8 + hp:9 + hp],
                                        scalar2=None, op0=ALU.add)
            return QT, KT

        qkt = [None] * NHP
        qkt[0] = emit_qk(0)

        # V projections for all heads (fills PE while first exps run)
        for tc_ in range(NKC):
            ps0 = pqk.tile([128, TOK], F32, tag="pqk")
            ps1 = pqk.tile([128, TOK], F32, tag="pqk")
            for dc in range(NDC):
                lhs = xk_sb[:, dc, tc_ * 128:(tc_ + 1) * 128]
                nc.tensor.matmul(ps0, lhs, wv_sb[:, dc, 0:512],
                                 start=(dc == 0), stop=(dc == NDC - 1))
                nc.tensor.matmul(ps1, lhs, wv_sb[:, dc, 512:1024],
                                 start=(dc == 0), stop=(dc == NDC - 1))
            nc.vector.scalar_tensor_tensor(
                out=V_sb[:, tc_ // 2, tc_ % 2, 0:8, 0:DV],
                in0=ps0[:].rearrange("p (h j) -> p h j", j=DV),
                scalar=SV, in1=bv_bc[:, 0:512].rearrange("p (h j) -> p h j", j=DV),
                op0=ALU.mult, op1=ALU.add)
            nc.vector.scalar_tensor_tensor(
                out=V_sb[:, tc_ // 2, tc_ % 2, 8:16, 0:DV],
                in0=ps1[:].rearrange("p (h j) -> p h j", j=DV),
                scalar=SV, in1=bv_bc[:, 512:1024].rearrange("p (h j) -> p h j", j=DV),
                op0=ALU.mult, op1=ALU.add)

        wvcm.close()  # wv region reusable by pools opened below
        ap = attn_cm.enter_context(tc.tile_pool(name="apool", bufs=2))
        npool = attn_cm.enter_context(tc.tile_pool(name="npool", bufs=1))

        for hp in range(NHP):
            # prefetch next head pair's projection weights + bulk weights
            if hp + 1 < NHP:
                wqk_t[hp + 1] = wqkp.tile([128, NDC, 256], BF16, tag="wqk",
                                         name=f"wqk_t{hp + 1}")
                nc.sync.dma_start(
                    out=wqk_t[hp + 1],
                    in_=_dram_chunked(wqk, 2 * D)[:, :, (hp + 1) * 256:(hp + 2) * 256])
            wo_t = wot.tile([128, D], BF16, tag="wo", name=f"wo_t{hp}")
            nc.sync.dma_start(out=wo_t, in_=_dram_chunked(wo, D)[:, hp, :])
            if hp < 4:  # w1 chunks 0..7: 0.5MB per hp at hp=0..3
                csl = slice(hp * 2 * 128, (hp + 1) * 2 * 128)
                nc.sync.dma_start(out=w1a_sb[:, :, csl],
                                  in_=_dram_chunked(w1, DFF)[:, :, csl])
            else:  # fp32 residual at hp=4..7
                qsl = slice((hp - 4) * 2, (hp - 3) * 2)
                nc.sync.dma_start(out=xres_sb[:, qsl, :],
                                  in_=_dram_chunked(xqTf, TOK)[:, qsl, :])

            QT, KT = qkt[hp]
            ctx0 = psC.tile([DV + 1, TOK], F32, tag="ctx")
            ctx1 = psC.tile([DV + 1, TOK], F32, tag="ctx")
            for half in range(2):
                a2 = ap.tile([128, NKC // 4, 2, 2 * TOK], FP8, tag="a2")
                # scores for 8 key chunks, both heads, batched (one PE mode)
                for kc in range(NKC // 2):
                    kcg = half * (NKC // 2) + kc
                    ksl = slice(kcg * 128, (kcg + 1) * 128)
                    s2 = psS.tile([128, 2 * TOK], F32, tag="s")
                    nc.tensor.matmul(s2[:, 0:TOK], KT[0:64, ksl], QT[0:64, :],
                                     start=True, stop=True)
                    nc.tensor.matmul(s2[:, TOK:2 * TOK], KT[64:128, ksl],
                                     QT[64:128, :], start=True, stop=True)
                    nc.scalar.activation(out=a2[:, kc // 2, kc % 2, :], in_=s2,
                                         func=AF.Exp, scale=1.0 / np.sqrt(DK),
                                         bias=lsa_t[:, 0:1])
                # ctx accumulation: fp8 DoubleRow over key pairs
                for kcp in range(NKC // 4):
                    kpg = half * (NKC // 4) + kcp
                    nc.tensor.matmul(ctx0, V_sb[:, kpg, :, 2 * hp, :],
                                     a2[:, kcp, :, 0:TOK],
                                     perf_mode=mybir.MatmulPerfMode.DoubleRow,
                                     start=(kpg == 0), stop=(kpg == NKC // 2 - 1))
                    nc.tensor.matmul(ctx1, V_sb[:, kpg, :, 2 * hp + 1, :],
                                     a2[:, kcp, :, TOK:2 * TOK],
                                     perf_mode=mybir.MatmulPerfMode.DoubleRow,
                                     start=(kpg == 0), stop=(kpg == NKC // 2 - 1))
            if hp + 1 < NHP:
                qkt[hp + 1] = emit_qk(hp + 1)
            # softmax normalization: divide by denominator (ones row of V)
            for j, ctx_ps in ((0, ctx0), (1, ctx1)):
                off = j * 64
                d_t = npool.tile([1, TOK], F32, tag="d")
                nc.vector.tensor_copy(out=d_t, in_=ctx_ps[DV:DV + 1, :])
                r_t = npool.tile([1, TOK], F32, tag="r")
                nc.vector.reciprocal_approx_fast(out=r_t, in_=d_t)
                rb_t = npool.tile([64, TOK], F32, tag="rb")
                nc.gpsimd.partition_broadcast(rb_t[:], r_t[:], channels=64)
                nc.vector.tensor_tensor(out=CT_sb[off:off + 64, hp, :],
                                        in0=ctx_ps[0:DV, :], in1=rb_t, op=ALU.mult)
            # Wo contribution of this head pair, accumulated in SBUF (GpSimd)
            for mc in range(NDC):
                psw = pqk.tile([128, TOK], F32, tag="pqk", name=f"psw{hp}_{mc}")
                nc.tensor.matmul(psw, wo_t[:, mc * 128:(mc + 1) * 128],
                                 CT_sb[:, hp, :], start=True, stop=True)
                nc.vector.tensor_tensor(out=woa_sb[:, mc, :], in0=woa_sb[:, mc, :],
                                        in1=psw, op=ALU.add)

        sqwarm = sp.tile([1, 1], F32)
        nc.scalar.activation(out=sqwarm, in_=eps_t, func=AF.Sqrt, bias=eps_t,
                             scale=1.0)

        attn_cm.close()  # free attention pools (xk, xq, V, wv, a2, KT/QT)

        bo_sb = sp.tile([128, 8], F32)
        nc.sync.dma_start(out=bo_sb, in_=_dram_chunked(bo, 1).rearrange("p c n -> p (c n)"))
        b1_sb = sp.tile([128, 32], F32)
        nc.sync.dma_start(out=b1_sb, in_=_dram_chunked(b1, 1).rearrange("p c n -> p (c n)"))
        b2_sb = sp.tile([128, 8], F32)
        nc.sync.dma_start(out=b2_sb, in_=_dram_chunked(b2, 1).rearrange("p c n -> p (c n)"))
        g1_sb = sp.tile([128, 8], F32)
        nc.sync.dma_start(out=g1_sb, in_=_dram_chunked(g1, 1).rearrange("p c n -> p (c n)"))
        be1_sb = sp.tile([128, 8], F32)
        nc.sync.dma_start(out=be1_sb, in_=_dram_chunked(be1, 1).rearrange("p c n -> p (c n)"))
        g2_sb = sp.tile([128, 8], F32)
        nc.sync.dma_start(out=g2_sb, in_=_dram_chunked(g2, 1).rearrange("p c n -> p (c n)"))
        be2_sb = sp.tile([128, 8], F32)
        nc.sync.dma_start(out=be2_sb, in_=_dram_chunked(be2, 1).rearrange("p c n -> p (c n)"))

        # ---------------- residual + LN1 (Wo already accumulated) ----------
        with ExitStack() as ph3:
            lnp = ph3.enter_context(tc.tile_pool(name="lnp", bufs=2))
            hp3 = ph3.enter_context(tc.tile_pool(name="hpool3", bufs=1))

            hT_sb = hp3.tile([128, NFC, TOK], BF16)
            ln1b_sb = hp3.tile([128, NDC, TOK], BF16)      # LN1 out (FFN rhs + residual)
            y1_sb = woa_sb

            w2p = ph3.enter_context(tc.tile_pool(name="w2p", bufs=3))
            psA = ExitStack()
            w1s = psA.enter_context(tc.tile_pool(name="w1s", bufs=3))
            psSt = psA.enter_context(tc.tile_pool(name="psSt", bufs=1, space="PSUM"))
            psbc = psA.enter_context(tc.tile_pool(name="psbc", bufs=1, space="PSUM"))
            psF = psA.enter_context(tc.tile_pool(name="psF", bufs=4, space="PSUM"))

            s_ps = psSt.tile([128, TOK], F32, tag="sum")
            q_ps = psSt.tile([128, TOK], F32, tag="sq")
            for mc in range(NDC):
                nc.vector.scalar_tensor_tensor(out=y1_sb[:, mc, :],
                                               in0=woa_sb[:, mc, :],
                                               scalar=bo_sb[:, mc:mc + 1],
                                               in1=xres_sb[:, mc, :],
                                               op0=ALU.add, op1=ALU.add)
                yb_t = lnp.tile([128, TOK], BF16, tag="yb")
                nc.vector.tensor_copy(out=yb_t, in_=y1_sb[:, mc, :])
                sq_t = lnp.tile([128, TOK], BF16, tag="sq")
                nc.scalar.square(out=sq_t, in_=y1_sb[:, mc, :])
                nc.tensor.matmul(s_ps, ones_mm, yb_t,
                                 start=(mc == 0), stop=(mc == NDC - 1))
                nc.tensor.matmul(q_ps, ones_mm, sq_t,
                                 start=(mc == 0), stop=(mc == NDC - 1))

            _ln_norm(nc, lnp, s_ps, q_ps, eps_t, y1_sb, g1_sb, be1_sb,
                     None, ln1b_sb, None, ones_r=ones_r, psbc=psbc)

            # FFN1: groups of 4 fc chunks; dc-ordered accumulation pipelines
            # with LN1 chunk production above
            w2_tiles = {}
            for mc in range(2):
                w2_tiles[mc] = w2p.tile([128, NFC, 128], BF16, tag="w2",
                                        name=f"w2_t{mc}")
                nc.sync.dma_start(out=w2_tiles[mc],
                                  in_=_dram_chunked(w2, D)[:, :, mc * 128:(mc + 1) * 128])
            for g in range(NFC // 4):
                pss = [psF.tile([128, TOK], F32, tag="f", name=f"psf{g}_{i}")
                       for i in range(4)]
                if g * 4 >= NW1PRE:
                    wt = w1s.tile([128, NDC, 4 * 128], BF16, tag="w1s")
                    nc.sync.dma_start(
                        out=wt,
                        in_=_dram_chunked(w1, DFF)[:, :, g * 512:(g + 1) * 512])
                for dc in range(NDC):
                    for f in range(4):
                        fc = g * 4 + f
                        if fc < NW1PRE:
                            lhs = w1a_sb[:, dc, fc * 128:(fc + 1) * 128]
                        else:
                            lhs = wt[:, dc, f * 128:(f + 1) * 128]
                        nc.tensor.matmul(pss[f], lhs, ln1b_sb[:, dc, :],
                                         start=(dc == 0), stop=(dc == NDC - 1))
                for f in range(4):
                    fc = g * 4 + f
                    nc.vector.tensor_scalar(out=hT_sb[:, fc, :], in0=pss[f],
                                            scalar1=b1_sb[:, fc:fc + 1],
                                            scalar2=0.0, op0=ALU.add, op1=ALU.max)

            psA.close()  # free Wo/FFN1 PSUM pools

            # ---------------- FFN2 + residual + LN2 ----------------
            psF2 = ph3.enter_context(tc.tile_pool(name="psF2", bufs=3, space="PSUM"))
            psSt2 = ph3.enter_context(tc.tile_pool(name="psSt2", bufs=1, space="PSUM"))
            psbc2 = ph3.enter_context(tc.tile_pool(name="psbc2", bufs=1, space="PSUM"))
            y2_sb = hp3.tile([128, NDC, TOK], F32)

            s2_ps = psSt2.tile([128, TOK], F32, tag="sum2")
            q2_ps = psSt2.tile([128, TOK], F32, tag="sq2")
            for mc in range(NDC):
                if mc in w2_tiles:
                    w2_t = w2_tiles[mc]
                else:
                    w2_t = w2p.tile([128, NFC, 128], BF16, tag="w2",
                                    name=f"w2_t{mc}")
                    nc.sync.dma_start(
                        out=w2_t,
                        in_=_dram_chunked(w2, D)[:, :, mc * 128:(mc + 1) * 128])
                ps = psF2.tile([128, TOK], F32, tag="f2")
                for fc in range(NFC):
                    nc.tensor.matmul(ps, w2_t[:, fc, :], hT_sb[:, fc, :],
                                     start=(fc == 0), stop=(fc == NFC - 1))
                nc.vector.scalar_tensor_tensor(out=y2_sb[:, mc, :], in0=ps,
                                               scalar=b2_sb[:, mc:mc + 1],
                                               in1=ln1b_sb[:, mc, :],
                                               op0=ALU.add, op1=ALU.add)
                yb_t = lnp.tile([128, TOK], BF16, tag="yb")
                nc.vector.tensor_copy(out=yb_t, in_=y2_sb[:, mc, :])
                sq_t = lnp.tile([128, TOK], BF16, tag="sq")
                nc.scalar.square(out=sq_t, in_=y2_sb[:, mc, :])
                nc.tensor.matmul(s2_ps, ones_mm, yb_t,
                                 start=(mc == 0), stop=(mc == NDC - 1))
                nc.tensor.matmul(q2_ps, ones_mm, sq_t,
                                 start=(mc == 0), stop=(mc == NDC - 1))

            _ln_norm(nc, lnp, s2_ps, q2_ps, eps_t, y2_sb, g2_sb, be2_sb,
                     None, None, (_dram_chunked(outT, TOK), nc),
                     ones_r=ones_r, psbc=psbc2)

    nc.compile()
    return nc


def _ln_norm(nc, lnp, s_ps, q_ps, eps_t, y_sb, g_sb, be_sb,
             out_f32, out_bf16, out_dma=None, ones_r=None, psbc=None):
    """Finish LayerNorm given accumulated sum/sumsq PSUM tiles (row 0)."""
    mean_t = lnp.tile([1, TOK], F32, tag="mean")
    nc.scalar.mul(out=mean_t, in_=s_ps[0:1, :], mul=1.0 / D)
    msq_t = lnp.tile([1, TOK], F32, tag="msq")
    nc.scalar.mul(out=msq_t, in_=q_ps[0:1, :], mul=1.0 / D)
    var_t = lnp.tile([1, TOK], F32, tag="var")
    nc.vector.tensor_tensor(out=var_t, in0=mean_t, in1=mean_t, op=ALU.mult)
    nc.vector.tensor_sub(out=var_t, in0=msq_t, in1=var_t)
    std_t = lnp.tile([1, TOK], F32, tag="std")
    nc.scalar.activation(out=std_t, in_=var_t, func=AF.Sqrt, bias=eps_t, scale=1.0)
    rstd_t = lnp.tile([1, TOK], F32, tag="rstd")
    nc.vector.reciprocal_approx_fast(out=rstd_t, in_=std_t)
    # broadcast mean/rstd across partitions with [1]-contraction matmuls
    # (Tensor engine is idle here; avoids GpSimd library thrash)
    mb_ps = psbc.tile([128, TOK], F32, tag="mb")
    nc.tensor.matmul(mb_ps, ones_r, mean_t, start=True, stop=True)
    rb_ps = psbc.tile([128, TOK], F32, tag="rb")
    nc.tensor.matmul(rb_ps, ones_r, rstd_t, start=True, stop=True)
    for mc in range(NDC):
        t1 = lnp.tile([128, TOK], BF16, tag="t1")
        nc.vector.tensor_sub(out=t1, in0=y_sb[:, mc, :], in1=mb_ps)
        t2 = lnp.tile([128, TOK], BF16, tag="t2")
        nc.vector.scalar_tensor_tensor(out=t2, in0=t1, scalar=g_sb[:, mc:mc + 1],
                                       in1=rb_ps, op0=ALU.mult, op1=ALU.mult)
        if out_f32 is not None or out_dma is not None:
            if out_f32 is not None:
                o_t = out_f32[:, mc, :]
            else:
                o_t = lnp.tile([128, TOK], BF16, tag="o3")
            nc.vector.tensor_scalar(out=o_t, in0=t2,
                                    scalar1=be_sb[:, mc:mc + 1], scalar2=None,
                                    op0=ALU.add)
            if out_dma is not None:
                dram, _nc = out_dma
                _nc.sync.dma_start(out=dram[:, mc, :], in_=o_t)
        if out_bf16 is not None:
            nc.vector.tensor_scalar(out=out_bf16[:, mc, :], in0=t2,
                                    scalar1=be_sb[:, mc:mc + 1], scalar2=None,
                                    op0=ALU.add)



_COMPILED = None
_LAST_IN_MAPS = None


def kernel(**inputs):
    global _COMPILED, _LAST_IN_MAPS
    ins = {k: np.asarray(v) for k, v in inputs.items()}
    x = _f32(ins["x"])
    Wq, bq = ins["Wq"], ins["bq"]
    Wk, bk = ins["Wk"], ins["bk"]
    Wv, bv = ins["Wv"], ins["bv"]
    Wo, bo = ins["Wo"], ins["bo"]
    W1, b1 = ins["W1"], ins["b1"]
    W2, b2 = ins["W2"], ins["b2"]
    g1, be1 = ins["g1"], ins["be1"]
    g2, be2 = ins["g2"], ins["be2"]

    wq2 = Wq.transpose(1, 0, 2).reshape(D, H * DK)
    wk2 = Wk.transpose(1, 0, 2).reshape(D, H * DK)
    # per-head-pair interleave: [q_hp (128) | k_hp (128)] blocks
    wqk = np.empty((D, 2 * D), np.float32)
    for hp in range(NHP):
        wqk[:, hp * 256:hp * 256 + 128] = wq2[:, hp * 128:(hp + 1) * 128]
        wqk[:, hp * 256 + 128:(hp + 1) * 256] = wk2[:, hp * 128:(hp + 1) * 128]
    bqkv = np.concatenate([bq.reshape(-1), bk.reshape(-1), bv.reshape(-1)])

    shared = {
        "wqk": _bf(wqk),
        "wv": _bf(Wv.transpose(1, 0, 2).reshape(D, H * DV)),
        "bqkv": _f32(bqkv.reshape(3 * D, 1)),
        "bv_row": _bf(bv.reshape(1, H * DV) * SV),
        "wo": _bf(Wo),
        "bo": _f32(bo.reshape(D, 1)),
        "w1": _bf(W1),
        "b1": _f32(b1.reshape(DFF, 1)),
        "w2": _bf(W2),
        "b2": _f32(b2.reshape(D, 1)),
        "g1": _f32(g1.reshape(D, 1)),
        "be1": _f32(be1.reshape(D, 1)),
        "g2": _f32(g2.reshape(D, 1)),
        "be2": _f32(be2.reshape(D, 1)),
    }

    in_maps = []
    for c in range(NCORES):
        b, qoff = c // 4, (c % 4) * TOK
        xb = x[b]                        # (S, D) fp32
        xkT = np.ascontiguousarray(xb.T)         # (D, S)
        xqT = np.ascontiguousarray(xb[qoff:qoff + TOK].T)  # (D, TOK)
        m = dict(shared)
        m["xkT"] = _bf(xkT)
        m["xqT"] = _bf(xqT)
        m["xqTf"] = _f32(xqT)
        in_maps.append(m)
    _LAST_IN_MAPS = in_maps

    if _COMPILED is None:
        _COMPILED = build()
    res = bass_utils.run_bass_kernel_spmd(_COMPILED, in_maps,
                                          core_ids=list(range(NCORES)))
    out = np.empty((B, S, D), np.float32)
    for c in range(NCORES):
        b, qoff = c // 4, (c % 4) * TOK
        out[b, qoff:qoff + TOK, :] = res.results[c]["outT"].T.astype(np.float32)
    return out


# revision 40
# speedup vs baseline: 1.0728x; 1.0177x over previous
"""Single transformer encoder layer on 8 Trainium2 NeuronCores.

Sharding: token-data-parallel, zero cross-core communication. Each core
owns 512 query tokens (4 cores per batch element) and computes K/V for
its whole batch locally, then attention, Wo, LN1, FFN, LN2.

Schedule (the point of this rewrite): QKV projections are computed
per-head-pair and software-pipelined with attention, so the ~147us of
softmax Exp on the Scalar engine hides under Tensor-engine matmuls
instead of gating them.  Score matmuls for the two heads of a pair are
emitted back-to-back on PE row-tiles (0,0)/(64,0) so they run
concurrently; scores for 8 key-chunks are batched between PE
mode-switches.  LayerNorm stats use [128,128] bf16 ones-matmuls (no
PE mode switch, 4x faster than fp32).  FFN relu/bias runs on the
Vector engine.  Weights (w1, wo) prefetch during attention.
"""

import sys

sys.path.insert(0, "/opt/trn_rl_repo")

import numpy as np
import ml_dtypes
from contextlib import ExitStack

import concourse.bass as bass
import concourse.mybir as mybir
import concourse.tile as tile
from concourse import bacc
import concourse.bass_utils as bass_utils

F32 = mybir.dt.float32
BF16 = mybir.dt.bfloat16
FP8 = mybir.dt.float8e4
AF = mybir.ActivationFunctionType
ALU = mybir.AluOpType

B, S, D = 2, 2048, 1024
H, DK, DV, DFF = 16, 64, 64, 4096
EPS = 1e-5
NCORES = 8
TOK = 512          # query tokens per core
SB = 2048          # batch tokens (K/V length)
NKC = SB // 128    # 16 key chunks
NDC = D // 128     # 8 feature chunks
NFC = DFF // 128   # 32 ffn chunks
NHP = H // 2       # 8 head pairs
NW1PRE = 8         # w1 fc-chunks prefetched during attention
SA = 2.0           # fp8 scale for softmax probs
SV = 32.0          # fp8 scale for V values
LOG_SA = float(np.log(SA))


def _bf(x):
    return np.ascontiguousarray(x.astype(ml_dtypes.bfloat16))


def _f32(x):
    return np.ascontiguousarray(x.astype(np.float32))


def _dram_chunked(t, ncols):
    """View a [R, ncols] DRAM tensor as [128, R//128, ncols]."""
    return t[:].rearrange("(c p) n -> p c n", p=128)


def build():
    nc = bacc.Bacc(name="encoder_layer", num_devices=NCORES)

    # ---- DRAM I/O ----
    xkT = nc.dram_tensor("xkT", [D, SB], BF16, kind="ExternalInput")
    xqT = nc.dram_tensor("xqT", [D, TOK], BF16, kind="ExternalInput")
    xqTf = nc.dram_tensor("xqTf", [D, TOK], F32, kind="ExternalInput")
    wqk = nc.dram_tensor("wqk", [D, 2 * D], BF16, kind="ExternalInput")
    wv = nc.dram_tensor("wv", [D, D], BF16, kind="ExternalInput")
    bqkv = nc.dram_tensor("bqkv", [3 * D, 1], F32, kind="ExternalInput")
    bv_row = nc.dram_tensor("bv_row", [1, D], BF16, kind="ExternalInput")
    wo = nc.dram_tensor("wo", [D, D], BF16, kind="ExternalInput")
    bo = nc.dram_tensor("bo", [D, 1], F32, kind="ExternalInput")
    w1 = nc.dram_tensor("w1", [D, DFF], BF16, kind="ExternalInput")
    b1 = nc.dram_tensor("b1", [DFF, 1], F32, kind="ExternalInput")
    w2 = nc.dram_tensor("w2", [DFF, D], BF16, kind="ExternalInput")
    b2 = nc.dram_tensor("b2", [D, 1], F32, kind="ExternalInput")
    g1 = nc.dram_tensor("g1", [D, 1], F32, kind="ExternalInput")
    be1 = nc.dram_tensor("be1", [D, 1], F32, kind="ExternalInput")
    g2 = nc.dram_tensor("g2", [D, 1], F32, kind="ExternalInput")
    be2 = nc.dram_tensor("be2", [D, 1], F32, kind="ExternalInput")
    outT = nc.dram_tensor("outT", [D, TOK], BF16, kind="ExternalOutput")

    with tile.TileContext(nc) as tc, ExitStack() as top:
        sp = top.enter_context(tc.tile_pool(name="smalls", bufs=1))

        ones_mm = sp.tile([128, 128], BF16)      # lhsT for column-sum matmuls
        nc.vector.memset(ones_mm, 1.0)
        ones_r = sp.tile([1, 128], F32)          # lhsT for partition-broadcast MMs
        nc.vector.memset(ones_r, 1.0)
        eps_t = sp.tile([1, 1], F32)
        nc.vector.memset(eps_t, EPS)
        lsa_t = sp.tile([128, 1], F32)
        nc.vector.memset(lsa_t, LOG_SA)
        bqkv_sb = sp.tile([128, 24], F32)
        nc.sync.dma_start(out=bqkv_sb,
                          in_=_dram_chunked(bqkv, 1).rearrange("p c n -> p (c n)"))

        # persistent activations (live across attention into the FFN)
        big = top.enter_context(tc.tile_pool(name="big", bufs=1))
        CT_sb = big.tile([128, NHP, TOK], BF16)        # ctx^T (heads on chunks)
        xres_sb = big.tile([128, NDC, TOK], F32)       # fp32 residual
        w1a_sb = big.tile([128, NDC, NW1PRE * 128], BF16)  # w1 fc 0..NW1PRE-1
        woa_sb = big.tile([128, NDC, TOK], F32)        # Wo output accumulator

        # attention-lifetime tensors (pool closed before the FFN phase)
        attn_cm = ExitStack()
        attx = attn_cm.enter_context(tc.tile_pool(name="attx", bufs=1))
        xk_sb = attx.tile([128, NDC, SB], BF16)        # x^T all batch tokens
        xq_sb = attx.tile([128, NDC, TOK], BF16)       # x^T own query tokens
        V_sb = attx.tile([128, NKC // 2, 2, H, DV + 1], FP8)  # V + ones col (x SV)

        # ---------------- QKV + attention, pipelined per head pair ----------------
        wqkp = attn_cm.enter_context(tc.tile_pool(name="wqkp", bufs=2))
        wot = attn_cm.enter_context(tc.tile_pool(name="wot", bufs=2))
        kqp = attn_cm.enter_context(tc.tile_pool(name="kqp", bufs=3))
        pqk = attn_cm.enter_context(tc.tile_pool(name="pqk", bufs=2, space="PSUM"))
        psS = attn_cm.enter_context(tc.tile_pool(name="psS", bufs=2, space="PSUM"))
        psC = attn_cm.enter_context(tc.tile_pool(name="psC", bufs=2, space="PSUM"))

        wvcm = ExitStack()
        wvp = wvcm.enter_context(tc.tile_pool(name="wvp", bufs=1))
        wv_sb = wvp.tile([128, NDC, D], BF16)

        # DMA order = issue order: everything that gates the first matmuls
        # goes first; K/V stream in by token-column quarters.
        wqk_t = [None] * NHP
        wqk_t[0] = wqkp.tile([128, NDC, 256], BF16, tag="wqk", name="wqk_t0")
        nc.scalar.dma_start(out=wqk_t[0],
                            in_=_dram_chunked(wqk, 2 * D)[:, :, 0:256])
        bv_bc = sp.tile([128, D], BF16)
        nc.scalar.dma_start(out=bv_bc, in_=bv_row[:].to_broadcast([128, D]))
        expwarm = sp.tile([1, 1], F32)
        nc.scalar.activation(out=expwarm, in_=eps_t, func=AF.Exp, scale=1.0)
        nc.gpsimd.dma_start(out=xq_sb, in_=_dram_chunked(xqT, TOK))
        nc.gpsimd.dma_start(out=wv_sb, in_=_dram_chunked(wv, D))
        for tq in range(4):
            nc.sync.dma_start(out=xk_sb[:, :, tq * TOK:(tq + 1) * TOK],
                              in_=_dram_chunked(xkT, SB)[:, :, tq * TOK:(tq + 1) * TOK])
        nc.vector.memset(V_sb[:, :, :, :, DV:DV + 1], SV)
        nc.vector.memset(woa_sb, 0.0)

        def emit_qk(hp):
            """Q^T and K^T projections for head pair hp -> fresh kq tiles."""
            QT = kqp.tile([128, TOK], BF16, tag="qt")
            KT = kqp.tile([128, SB], BF16, tag="kt")
            w = wqk_t[hp]
            ps = pqk.tile([128, TOK], F32, tag="pqk")
            for dc in range(NDC):
                nc.tensor.matmul(ps, w[:, dc, 0:128], xq_sb[:, dc, :],
                                 start=(dc == 0), stop=(dc == NDC - 1))
            nc.vector.tensor_scalar(out=QT, in0=ps,
                                    scalar1=bqkv_sb[:, hp:hp + 1], scalar2=None,
                                    op0=ALU.add)
            for tt in range(SB // TOK):
                ps = pqk.tile([128, TOK], F32, tag="pqk")
                for dc in range(NDC):
                    nc.tensor.matmul(ps, w[:, dc, 128:256],
                                     xk_sb[:, dc, tt * TOK:(tt + 1) * TOK],
                                     start=(dc == 0), stop=(dc == NDC - 1))
                nc.vector.tensor_scalar(out=KT[:, tt * TOK:(tt + 1) * TOK],
                                        in0=ps, scalar1=bqkv_sb[:, 8 + hp:9 + hp],
                                        scalar2=None, op0=ALU.add)
            return QT, KT

        qkt = [None] * NHP
        qkt[0] = emit_qk(0)

        # V projections for all heads (fills PE while first exps run)
        for tc_ in range(NKC):
            ps0 = pqk.tile([128, TOK], F32, tag="pqk")
            ps1 = pqk.tile([128, TOK], F32, tag="pqk")
            for dc in range(NDC):
                lhs = xk_sb[:, dc, tc_ * 128:(tc_ + 1) * 128]
                nc.tensor.matmul(ps0, lhs, wv_sb[:, dc, 0:512],
                                 start=(dc == 0), stop=(dc == NDC - 1))
                nc.tensor.matmul(ps1, lhs, wv_sb[:, dc, 512:1024],
                                 start=(dc == 0), stop=(dc == NDC - 1))
            nc.vector.scalar_tensor_tensor(
                out=V_sb[:, tc_ // 2, tc_ % 2, 0:8, 0:DV],
                in0=ps0[:].rearrange("p (h j) -> p h j", j=DV),
                scalar=SV, in1=bv_bc[:, 0:512].rearrange("p (h j) -> p h j", j=DV),
                op0=ALU.mult, op1=ALU.add)
            nc.vector.scalar_tensor_tensor(
                out=V_sb[:, tc_ // 2, tc_ % 2, 8:16, 0:DV],
                in0=ps1[:].rearrange("p (h j) -> p h j", j=DV),
                scalar=SV, in1=bv_bc[:, 512:1024].rearrange("p (h j) -> p h j", j=DV),
                op0=ALU.mult, op1=ALU.add)

        wvcm.close()  # wv region reusable by pools opened below
        ap = attn_cm.enter_context(tc.tile_pool(name="apool", bufs=2))
        npool = attn_cm.enter_context(tc.tile_pool(name="npool", bufs=1))

        for hp in range(NHP):
            # prefetch next head pair's projection weights + bulk weights
            if hp + 1 < NHP:
                wqk_t[hp + 1] = wqkp.tile([128, NDC, 256], BF16, tag="wqk",
                                         name=f"wqk_t{hp + 1}")
                nc.sync.dma_start(
                    out=wqk_t[hp + 1],
                    in_=_dram_chunked(wqk, 2 * D)[:, :, (hp + 1) * 256:(hp + 2) * 256])
            wo_t = wot.tile([128, D], BF16, tag="wo", name=f"wo_t{hp}")
            nc.sync.dma_start(out=wo_t, in_=_dram_chunked(wo, D)[:, hp, :])
            if hp < 4:  # w1 chunks 0..7: 0.5MB per hp at hp=0..3
                csl = slice(hp * 2 * 128, (hp + 1) * 2 * 128)
                nc.sync.dma_start(out=w1a_sb[:, :, csl],
                                  in_=_dram_chunked(w1, DFF)[:, :, csl])
            else:  # fp32 residual at hp=4..7
                qsl = slice((hp - 4) * 2, (hp - 3) * 2)
                nc.sync.dma_start(out=xres_sb[:, qsl, :],
                                  in_=_dram_chunked(xqTf, TOK)[:, qsl, :])

            QT, KT = qkt[hp]
            ctx0 = psC.tile([DV + 1, TOK], F32, tag="ctx")
            ctx1 = psC.tile([DV + 1, TOK], F32, tag="ctx")
            for half in range(2):
                a2 = ap.tile([128, NKC // 4, 2, 2 * TOK], FP8, tag="a2")
                # scores for 8 key chunks, both heads, batched (one PE mode)
                for kc in range(NKC // 2):
                    kcg = half * (NKC // 2) + kc
                    ksl = slice(kcg * 128, (kcg + 1) * 128)
                    s2 = psS.tile([128, 2 * TOK], F32, tag="s")
                    nc.tensor.matmul(s2[:, 0:TOK], KT[0:64, ksl], QT[0:64, :],
                                     start=True, stop=True)
                    nc.tensor.matmul(s2[:, TOK:2 * TOK], KT[64:128, ksl],
                                     QT[64:128, :], start=True, stop=True)
                    nc.scalar.activation(out=a2[:, kc // 2, kc % 2, :], in_=s2,
                                         func=AF.Exp, scale=1.0 / np.sqrt(DK),
                                         bias=lsa_t[:, 0:1])
                # ctx accumulation: fp8 DoubleRow over key pairs
                for kcp in range(NKC // 4):
                    kpg = half * (NKC // 4) + kcp
                    nc.tensor.matmul(ctx0, V_sb[:, kpg, :, 2 * hp, :],
                                     a2[:, kcp, :, 0:TOK],
                                     perf_mode=mybir.MatmulPerfMode.DoubleRow,
                                     start=(kpg == 0), stop=(kpg == NKC // 2 - 1))
                    nc.tensor.matmul(ctx1, V_sb[:, kpg, :, 2 * hp + 1, :],
                                     a2[:, kcp, :, TOK:2 * TOK],
                                     perf_mode=mybir.MatmulPerfMode.DoubleRow,
                                     start=(kpg == 0), stop=(kpg == NKC // 2 - 1))
            if hp + 1 < NHP:
                qkt[hp + 1] = emit_qk(hp + 1)
            # softmax normalization: divide by denominator (ones row of V)
            for j, ctx_ps in ((0, ctx0), (1, ctx1)):
                off = j * 64
                d_t = npool.tile([1, TOK], F32, tag="d")
                nc.vector.tensor_copy(out=d_t, in_=ctx_ps[DV:DV + 1, :])
                r_t = npool.tile([1, TOK], F32, tag="r")
                nc.vector.reciprocal_approx_fast(out=r_t, in_=d_t)
                rb_t = npool.tile([64, TOK], F32, tag="rb")
                nc.gpsimd.partition_broadcast(rb_t[:], r_t[:], channels=64)
                nc.vector.tensor_tensor(out=CT_sb[off:off + 64, hp, :],
                                        in0=ctx_ps[0:DV, :], in1=rb_t, op=ALU.mult)
            # Wo contribution of this head pair, accumulated in SBUF (GpSimd)
            for mc in range(NDC):
                psw = pqk.tile([128, TOK], F32, tag="pqk", name=f"psw{hp}_{mc}")
                nc.tensor.matmul(psw, wo_t[:, mc * 128:(mc + 1) * 128],
                                 CT_sb[:, hp, :], start=True, stop=True)
                nc.vector.tensor_tensor(out=woa_sb[:, mc, :], in0=woa_sb[:, mc, :],
                                        in1=psw, op=ALU.add)

        sqwarm = sp.tile([1, 1], F32)
        nc.scalar.activation(out=sqwarm, in_=eps_t, func=AF.Sqrt, bias=eps_t,
                             scale=1.0)

        attn_cm.close()  # free attention pools (xk, xq, V, wv, a2, KT/QT)

        bo_sb = sp.tile([128, 8], F32)
        nc.sync.dma_start(out=bo_sb, in_=_dram_chunked(bo, 1).rearrange("p c n -> p (c n)"))
        b1_sb = sp.tile([128, 32], F32)
        nc.sync.dma_start(out=b1_sb, in_=_dram_chunked(b1, 1).rearrange("p c n -> p (c n)"))
        b2_sb = sp.tile([128, 8], F32)
        nc.sync.dma_start(out=b2_sb, in_=_dram_chunked(b2, 1).rearrange("p c n -> p (c n)"))
        g1_sb = sp.tile([128, 8], F32)
        nc.sync.dma_start(out=g1_sb, in_=_dram_chunked(g1, 1).rearrange("p c n -> p (c n)"))
        be1_sb = sp.tile([128, 8], F32)
        nc.sync.dma_start(out=be1_sb, in_=_dram_chunked(be1, 1).rearrange("p c n -> p (c n)"))
        g2_sb = sp.tile([128, 8], F32)
        nc.sync.dma_start(out=g2_sb, in_=_dram_chunked(g2, 1).rearrange("p c n -> p (c n)"))
        be2_sb = sp.tile([128, 8], F32)
        nc.sync.dma_start(out=be2_sb, in_=_dram_chunked(be2, 1).rearrange("p c n -> p (c n)"))

        # ---------------- residual + LN1 (Wo already accumulated) ----------
        with ExitStack() as ph3:
            lnp = ph3.enter_context(tc.tile_pool(name="lnp", bufs=2))
            hp3 = ph3.enter_context(tc.tile_pool(name="hpool3", bufs=1))

            hT_sb = hp3.tile([128, NFC, TOK], BF16)
            ln1b_sb = hp3.tile([128, NDC, TOK], BF16)      # LN1 out (FFN rhs + residual)
            y1_sb = woa_sb

            w2p = ph3.enter_context(tc.tile_pool(name="w2p", bufs=3))
            psA = ExitStack()
            w1s = psA.enter_context(tc.tile_pool(name="w1s", bufs=3))
            psSt = psA.enter_context(tc.tile_pool(name="psSt", bufs=1, space="PSUM"))
            psbc = psA.enter_context(tc.tile_pool(name="psbc", bufs=1, space="PSUM"))
            psF = psA.enter_context(tc.tile_pool(name="psF", bufs=4, space="PSUM"))

            s_ps = psSt.tile([128, TOK], F32, tag="sum")
            q_ps = psSt.tile([128, TOK], F32, tag="sq")
            for mc in range(NDC):
                nc.vector.scalar_tensor_tensor(out=y1_sb[:, mc, :],
                                               in0=woa_sb[:, mc, :],
                                               scalar=bo_sb[:, mc:mc + 1],
                                               in1=xres_sb[:, mc, :],
                                               op0=ALU.add, op1=ALU.add)
                yb_t = lnp.tile([128, TOK], BF16, tag="yb")
                nc.vector.tensor_copy(out=yb_t, in_=y1_sb[:, mc, :])
                sq_t = lnp.tile([128, TOK], BF16, tag="sq")
                nc.scalar.square(out=sq_t, in_=y1_sb[:, mc, :])
                nc.tensor.matmul(s_ps, ones_mm, yb_t,
                                 start=(mc == 0), stop=(mc == NDC - 1))
                nc.tensor.matmul(q_ps, ones_mm, sq_t,
                                 start=(mc == 0), stop=(mc == NDC - 1))

            _ln_norm(nc, lnp, s_ps, q_ps, eps_t, y1_sb, g1_sb, be1_sb,
                     None, ln1b_sb, None, ones_r=ones_r, psbc=psbc)

            # FFN1: groups of 4 fc chunks; dc-ordered accumulation pipelines
            # with LN1 chunk production above
            w2_tiles = {}
            for mc in range(2):
                w2_tiles[mc] = w2p.tile([128, NFC, 128], BF16, tag="w2",
                                        name=f"w2_t{mc}")
                nc.sync.dma_start(out=w2_tiles[mc],
                                  in_=_dram_chunked(w2, D)[:, :, mc * 128:(mc + 1) * 128])
            for g in range(NFC // 4):
                pss = [psF.tile([128, TOK], F32, tag="f", name=f"psf{g}_{i}")
                       for i in range(4)]
                if g * 4 >= NW1PRE:
                    wt = w1s.tile([128, NDC, 4 * 128], BF16, tag="w1s")
                    nc.sync.dma_start(
                        out=wt,
                        in_=_dram_chunked(w1, DFF)[:, :, g * 512:(g + 1) * 512])
                for dc in range(NDC):
                    for f in range(4):
                        fc = g * 4 + f
                        if fc < NW1PRE:
                            lhs = w1a_sb[:, dc, fc * 128:(fc + 1) * 128]
                        else:
                            lhs = wt[:, dc, f * 128:(f + 1) * 128]
                        nc.tensor.matmul(pss[f], lhs, ln1b_sb[:, dc, :],
                                         start=(dc == 0), stop=(dc == NDC - 1))
                for f in range(4):
                    fc = g * 4 + f
                    nc.vector.tensor_scalar(out=hT_sb[:, fc, :], in0=pss[f],
                                            scalar1=b1_sb[:, fc:fc + 1],
                                            scalar2=0.0, op0=ALU.add, op1=ALU.max)

            psA.close()  # free Wo/FFN1 PSUM pools

            # ---------------- FFN2 + residual + LN2 ----------------
            psF2 = ph3.enter_context(tc.tile_pool(name="psF2", bufs=3, space="PSUM"))
            psSt2 = ph3.enter_context(tc.tile_pool(name="psSt2", bufs=1, space="PSUM"))
            psbc2 = ph3.enter_context(tc.tile_pool(name="psbc2", bufs=1, space="PSUM"))
            y2_sb = hp3.tile([128, NDC, TOK], F32)

            s2_ps = psSt2.tile([128, TOK], F32, tag="sum2")
            q2_ps = psSt2.tile([128, TOK], F32, tag="sq2")
            for mc in range(NDC):
                if mc in w2_tiles:
                    w2_t = w2_tiles[mc]
                else:
                    w2_t = w2p.tile([128, NFC, 128], BF16, tag="w2",
                                    name=f"w2_t{mc}")
                    nc.sync.dma_start(
                        out=w2_t,
                        in_=_dram_chunked(w2, D)[:, :, mc * 128:(mc + 1) * 128])
                ps = psF2.tile([128, TOK], F32, tag="f2")
                for fc in range(NFC):
                    nc.tensor.matmul(ps, w2_t[:, fc, :], hT_sb[:, fc, :],
                                     start=(fc == 0), stop=(fc == NFC - 1))
                nc.vector.scalar_tensor_tensor(out=y2_sb[:, mc, :], in0=ps,
                                               scalar=b2_sb[:, mc:mc + 1],
                                               in1=ln1b_sb[:, mc, :],
                                               op0=ALU.add, op1=ALU.add)
                yb_t = lnp.tile([128, TOK], BF16, tag="yb")
                nc.vector.tensor_copy(out=yb_t, in_=y2_sb[:, mc, :])
                sq_t = lnp.tile([128, TOK], BF16, tag="sq")
                nc.scalar.square(out=sq_t, in_=y2_sb[:, mc, :])
                nc.tensor.matmul(s2_ps, ones_mm, yb_t,
                                 start=(mc == 0), stop=(mc == NDC - 1))
                nc.tensor.matmul(q2_ps, ones_mm, sq_t,
                                 start=(mc == 0), stop=(mc == NDC - 1))

            _ln_norm(nc, lnp, s2_ps, q2_ps, eps_t, y2_sb, g2_sb, be2_sb,
                     None, None, (_dram_chunked(outT, TOK), nc),
                     ones_r=ones_r, psbc=psbc2)

    nc.compile()
    return nc


def _ln_norm(nc, lnp, s_ps, q_ps, eps_t, y_sb, g_sb, be_sb,
             out_f32, out_bf16, out_dma=None, ones_r=None, psbc=None):
    """Finish LayerNorm given accumulated sum/sumsq PSUM tiles (row 0)."""
    mean_t = lnp.tile([1, TOK], F32, tag="mean")
    nc.scalar.mul(out=mean_t, in_=s_ps[0:1, :], mul=1.0 / D)
    msq_t = lnp.tile([1, TOK], F32, tag="msq")
    nc.scalar.mul(out=msq_t, in_=q_ps[0:1, :], mul=1.0 / D)
    var_t = lnp.tile([1, TOK], F32, tag="var")
    nc.vector.tensor_tensor(out=var_t, in0=mean_t, in1=mean_t, op=ALU.mult)
    nc.vector.tensor_sub(out=var_t, in0=msq_t, in1=var_t)
    std_t = lnp.tile([1, TOK], F32, tag="std")
    nc.scalar.activation(out=std_t, in_=var_t, func=AF.Sqrt, bias=eps_t, scale=1.0)
    rstd_t = lnp.tile([1, TOK], F32, tag="rstd")
    nc.vector.reciprocal_approx_fast(out=rstd_t, in_=std_t)
    # broadcast mean/rstd across partitions with [1]-contraction matmuls
    # (Tensor engine is idle here; avoids GpSimd library thrash)
    mb_ps = psbc.tile([128, TOK], F32, tag="mb")
    nc.tensor.matmul(mb_ps, ones_r, mean_t, start=True, stop=True)
    rb_ps = psbc.tile([128, TOK], F32, tag="rb")
    nc.tensor.matmul(rb_ps, ones_r, rstd_t, start=True, stop=True)
    for mc in range(NDC):
        t1 = lnp.tile([128, TOK], BF16, tag="t1")
        nc.vector.tensor_sub(out=t1, in0=y_sb[:, mc, :], in1=mb_ps)
        t2 = lnp.tile([128, TOK], BF16, tag="t2")
        nc.vector.scalar_tensor_tensor(out=t2, in0=t1, scalar=g_sb[:, mc:mc + 1],
                                       in1=rb_ps, op0=ALU.mult, op1=ALU.mult)
        if out_f32 is not None or out_dma is not None:
            if out_f32 is not None:
                o_t = out_f32[:, mc, :]
            else:
                o_t = lnp.tile([128, TOK], BF16, tag="o3")
            nc.vector.tensor_scalar(out=o_t, in0=t2,
                                    scalar1=be_sb[:, mc:mc + 1], scalar2=None,
                                    op0=ALU.add)
            if out_dma is not None:
                dram, _nc = out_dma
                _nc.sync.dma_start(out=dram[:, mc, :], in_=o_t)
        if out_bf16 is not None:
            nc.vector.tensor_scalar(out=out_bf16[:, mc, :], in0=t2,
                                    scalar1=be_sb[:, mc:mc + 1], scalar2=None,
                                    op0=ALU.add)



_COMPILED = None
_LAST_IN_MAPS = None


def kernel(**inputs):
    global _COMPILED, _LAST_IN_MAPS
    ins = {k: np.asarray(v) for k, v in inputs.items()}
    x = _f32(ins["x"])
    Wq, bq = ins["Wq"], ins["bq"]
    Wk, bk = ins["Wk"], ins["bk"]
    Wv, bv = ins["Wv"], ins["bv"]
    Wo, bo = ins["Wo"], ins["bo"]
    W1, b1 = ins["W1"], ins["b1"]
    W2, b2 = ins["W2"], ins["b2"]
    g1, be1 = ins["g1"], ins["be1"]
    g2, be2 = ins["g2"], ins["be2"]

    wq2 = Wq.transpose(1, 0, 2).reshape(D, H * DK)
    wk2 = Wk.transpose(1, 0, 2).reshape(D, H * DK)
    # per-head-pair interleave: [q_hp (128) | k_hp (128)] blocks
    wqk = np.empty((D, 2 * D), np.float32)
    for hp in range(NHP):
        wqk[:, hp * 256:hp * 256 + 128] = wq2[:, hp * 128:(hp + 1) * 128]
        wqk[:, hp * 256 + 128:(hp + 1) * 256] = wk2[:, hp * 128:(hp + 1) * 128]
    bqkv = np.concatenate([bq.reshape(-1), bk.reshape(-1), bv.reshape(-1)])

    shared = {
        "wqk": _bf(wqk),
        "wv": _bf(Wv.transpose(1, 0, 2).reshape(D, H * DV)),
        "bqkv": _f32(bqkv.reshape(3 * D, 1)),
        "bv_row": _bf(bv.reshape(1, H * DV) * SV),
        "wo": _bf(Wo),
        "bo": _f32(bo.reshape(D, 1)),
        "w1": _bf(W1),
        "b1": _f32(b1.reshape(DFF, 1)),
        "w2": _bf(W2),
        "b2": _f32(b2.reshape(D, 1)),
        "g1": _f32(g1.reshape(D, 1)),
        "be1": _f32(be1.reshape(D, 1)),
        "g2": _f32(g2.reshape(D, 1)),
        "be2": _f32(be2.reshape(D, 1)),
    }

    in_maps = []
    for c in range(NCORES):
        b, qoff = c // 4, (c % 4) * TOK
        xb = x[b]                        # (S, D) fp32
        xkT = np.ascontiguousarray(xb.T)         # (D, S)
        xqT = np.ascontiguousarray(xb[qoff:qoff + TOK].T)  # (D, TOK)
        m = dict(shared)
        m["xkT"] = _bf(xkT)
        m["xqT"] = _bf(xqT)
        m["xqTf"] = _f32(xqT)
        in_maps.append(m)
    _LAST_IN_MAPS = in_maps

    if _COMPILED is None:
        _COMPILED = build()
    res = bass_utils.run_bass_kernel_spmd(_COMPILED, in_maps,
                                          core_ids=list(range(NCORES)))
    out = np.empty((B, S, D), np.float32)
    for c in range(NCORES):
        b, qoff = c // 4, (c % 4) * TOK
        out[b, qoff:qoff + TOK, :] = res.results[c]["outT"].T.astype(np.float32)
    return out


# revision 41
# speedup vs baseline: 1.1130x; 1.0375x over previous
"""Single transformer encoder layer on 8 Trainium2 NeuronCores.

Sharding: token-data-parallel, zero cross-core communication. Each core
owns 512 query tokens (4 cores per batch element) and computes K/V for
its whole batch locally, then attention, Wo, LN1, FFN, LN2.

Schedule (the point of this rewrite): QKV projections are computed
per-head-pair and software-pipelined with attention, so the ~147us of
softmax Exp on the Scalar engine hides under Tensor-engine matmuls
instead of gating them.  Score matmuls for the two heads of a pair are
emitted back-to-back on PE row-tiles (0,0)/(64,0) so they run
concurrently; scores for 8 key-chunks are batched between PE
mode-switches.  LayerNorm stats use [128,128] bf16 ones-matmuls (no
PE mode switch, 4x faster than fp32).  FFN relu/bias runs on the
Vector engine.  Weights (w1, wo) prefetch during attention.
"""

import sys

sys.path.insert(0, "/opt/trn_rl_repo")

import numpy as np
import ml_dtypes
from contextlib import ExitStack

import concourse.bass as bass
import concourse.mybir as mybir
import concourse.tile as tile
from concourse import bacc
import concourse.bass_utils as bass_utils

F32 = mybir.dt.float32
BF16 = mybir.dt.bfloat16
FP8 = mybir.dt.float8e4
AF = mybir.ActivationFunctionType
ALU = mybir.AluOpType

B, S, D = 2, 2048, 1024
H, DK, DV, DFF = 16, 64, 64, 4096
EPS = 1e-5
NCORES = 8
TOK = 512          # query tokens per core
SB = 2048          # batch tokens (K/V length)
NKC = SB // 128    # 16 key chunks
NDC = D // 128     # 8 feature chunks
NFC = DFF // 128   # 32 ffn chunks
NHP = H // 2       # 8 head pairs
NW1PRE = 8         # w1 fc-chunks prefetched during attention
SA = 2.0           # fp8 scale for softmax probs
SV = 32.0          # fp8 scale for V values
LOG_SA = float(np.log(SA))


def _bf(x):
    return np.ascontiguousarray(x.astype(ml_dtypes.bfloat16))


def _f32(x):
    return np.ascontiguousarray(x.astype(np.float32))


def _dram_chunked(t, ncols):
    """View a [R, ncols] DRAM tensor as [128, R//128, ncols]."""
    return t[:].rearrange("(c p) n -> p c n", p=128)


def build():
    nc = bacc.Bacc(name="encoder_layer", num_devices=NCORES)

    # ---- DRAM I/O ----
    xkT = nc.dram_tensor("xkT", [D, SB], BF16, kind="ExternalInput")
    xqT = nc.dram_tensor("xqT", [D, TOK], BF16, kind="ExternalInput")
    xqTf = nc.dram_tensor("xqTf", [D, TOK], F32, kind="ExternalInput")
    wqk = nc.dram_tensor("wqk", [D, 2 * D], BF16, kind="ExternalInput")
    wv = nc.dram_tensor("wv", [D, D], BF16, kind="ExternalInput")
    bqkv = nc.dram_tensor("bqkv", [3 * D, 1], F32, kind="ExternalInput")
    bv_row = nc.dram_tensor("bv_row", [1, D], BF16, kind="ExternalInput")
    wo = nc.dram_tensor("wo", [D, D], BF16, kind="ExternalInput")
    bo = nc.dram_tensor("bo", [D, 1], F32, kind="ExternalInput")
    w1 = nc.dram_tensor("w1", [D, DFF], BF16, kind="ExternalInput")
    b1 = nc.dram_tensor("b1", [DFF, 1], F32, kind="ExternalInput")
    w2 = nc.dram_tensor("w2", [DFF, D], BF16, kind="ExternalInput")
    b2 = nc.dram_tensor("b2", [D, 1], F32, kind="ExternalInput")
    g1 = nc.dram_tensor("g1", [D, 1], F32, kind="ExternalInput")
    be1 = nc.dram_tensor("be1", [D, 1], F32, kind="ExternalInput")
    g2 = nc.dram_tensor("g2", [D, 1], F32, kind="ExternalInput")
    be2 = nc.dram_tensor("be2", [D, 1], F32, kind="ExternalInput")
    outT = nc.dram_tensor("outT", [D, TOK], BF16, kind="ExternalOutput")

    with tile.TileContext(nc) as tc, ExitStack() as top:
        sp = top.enter_context(tc.tile_pool(name="smalls", bufs=1))

        ones_mm = sp.tile([128, 128], BF16)      # lhsT for column-sum matmuls
        nc.vector.memset(ones_mm, 1.0)
        ones_r = sp.tile([1, 128], F32)          # lhsT for partition-broadcast MMs
        nc.vector.memset(ones_r, 1.0)
        eps_t = sp.tile([1, 1], F32)
        nc.vector.memset(eps_t, EPS)
        lsa_t = sp.tile([128, 1], F32)
        nc.vector.memset(lsa_t, LOG_SA)
        bqkv_sb = sp.tile([128, 24], F32)
        nc.sync.dma_start(out=bqkv_sb,
                          in_=_dram_chunked(bqkv, 1).rearrange("p c n -> p (c n)"))

        # persistent activations (live across attention into the FFN)
        big = top.enter_context(tc.tile_pool(name="big", bufs=1))
        CT_sb = big.tile([128, NHP, TOK], BF16)        # ctx^T (heads on chunks)
        xres_sb = big.tile([128, NDC, TOK], F32)       # fp32 residual
        w1a_sb = big.tile([128, NDC, NW1PRE * 128], BF16)  # w1 fc 0..NW1PRE-1
        woa_sb = big.tile([128, NDC, TOK], F32)        # Wo output accumulator

        # attention-lifetime tensors (pool closed before the FFN phase)
        attn_cm = ExitStack()
        attx = attn_cm.enter_context(tc.tile_pool(name="attx", bufs=1))
        xk_sb = attx.tile([128, NDC, SB], BF16)        # x^T all batch tokens
        xq_sb = attx.tile([128, NDC, TOK], BF16)       # x^T own query tokens
        V_sb = attx.tile([128, NKC // 2, 2, H, DV + 1], FP8)  # V + ones col (x SV)

        # ---------------- QKV + attention, pipelined per head pair ----------------
        wqkp = attn_cm.enter_context(tc.tile_pool(name="wqkp", bufs=2))
        wot = attn_cm.enter_context(tc.tile_pool(name="wot", bufs=2))
        kqp = attn_cm.enter_context(tc.tile_pool(name="kqp", bufs=3))
        pqk = attn_cm.enter_context(tc.tile_pool(name="pqk", bufs=2, space="PSUM"))
        psS = attn_cm.enter_context(tc.tile_pool(name="psS", bufs=2, space="PSUM"))
        psC = attn_cm.enter_context(tc.tile_pool(name="psC", bufs=2, space="PSUM"))

        wvcm = ExitStack()
        wvp = wvcm.enter_context(tc.tile_pool(name="wvp", bufs=1))
        wv_sb = wvp.tile([128, NDC, D], BF16)

        # DMA order = issue order: everything that gates the first matmuls
        # goes first; K/V stream in by token-column quarters.
        wqk_t = [None] * NHP
        wqk_t[0] = wqkp.tile([128, NDC, 256], BF16, tag="wqk", name="wqk_t0")
        nc.scalar.dma_start(out=wqk_t[0],
                            in_=_dram_chunked(wqk, 2 * D)[:, :, 0:256])
        bv_bc = sp.tile([128, D], BF16)
        nc.scalar.dma_start(out=bv_bc, in_=bv_row[:].to_broadcast([128, D]))
        expwarm = sp.tile([1, 1], F32)
        nc.scalar.activation(out=expwarm, in_=eps_t, func=AF.Exp, scale=1.0)
        nc.gpsimd.dma_start(out=xq_sb, in_=_dram_chunked(xqT, TOK))
        nc.gpsimd.dma_start(out=wv_sb, in_=_dram_chunked(wv, D))
        for tq in range(4):
            nc.sync.dma_start(out=xk_sb[:, :, tq * TOK:(tq + 1) * TOK],
                              in_=_dram_chunked(xkT, SB)[:, :, tq * TOK:(tq + 1) * TOK])
        nc.vector.memset(V_sb[:, :, :, :, DV:DV + 1], SV)
        nc.vector.memset(woa_sb, 0.0)

        def emit_qk(hp):
            """Q^T and K^T projections for head pair hp -> fresh kq tiles."""
            QT = kqp.tile([128, TOK], BF16, tag="qt")
            KT = kqp.tile([128, SB], BF16, tag="kt")
            w = wqk_t[hp]
            ps = pqk.tile([128, TOK], F32, tag="pqk")
            for dc in range(NDC):
                nc.tensor.matmul(ps, w[:, dc, 0:128], xq_sb[:, dc, :],
                                 start=(dc == 0), stop=(dc == NDC - 1))
            nc.vector.tensor_scalar(out=QT, in0=ps,
                                    scalar1=bqkv_sb[:, hp:hp + 1], scalar2=None,
                                    op0=ALU.add)
            for tt in range(SB // TOK):
                ps = pqk.tile([128, TOK], F32, tag="pqk")
                for dc in range(NDC):
                    nc.tensor.matmul(ps, w[:, dc, 128:256],
                                     xk_sb[:, dc, tt * TOK:(tt + 1) * TOK],
                                     start=(dc == 0), stop=(dc == NDC - 1))
                nc.vector.tensor_scalar(out=KT[:, tt * TOK:(tt + 1) * TOK],
                                        in0=ps, scalar1=bqkv_sb[:, 8 + hp:9 + hp],
                                        scalar2=None, op0=ALU.add)
            return QT, KT

        qkt = [None] * NHP
        qkt[0] = emit_qk(0)

        # V projections for all heads (fills PE while first exps run).
        # Uses the scores PSUM pool (idle until attention) so pqk's two
        # buffers stay free for Q/K and chunk N+1's matmuls never wait on
        # chunk N's drain.
        for tc_ in range(NKC):
            psv = psS.tile([128, 2 * TOK], F32, tag="s", name=f"psv{tc_}")
            for dc in range(NDC):
                lhs = xk_sb[:, dc, tc_ * 128:(tc_ + 1) * 128]
                nc.tensor.matmul(psv[:, 0:TOK], lhs, wv_sb[:, dc, 0:512],
                                 start=(dc == 0), stop=(dc == NDC - 1))
                nc.tensor.matmul(psv[:, TOK:2 * TOK], lhs, wv_sb[:, dc, 512:1024],
                                 start=(dc == 0), stop=(dc == NDC - 1))
            nc.vector.scalar_tensor_tensor(
                out=V_sb[:, tc_ // 2, tc_ % 2, 0:16, 0:DV],
                in0=psv[:].rearrange("p (h j) -> p h j", j=DV),
                scalar=SV, in1=bv_bc[:, 0:1024].rearrange("p (h j) -> p h j", j=DV),
                op0=ALU.mult, op1=ALU.add)

        wvcm.close()  # wv region reusable by pools opened below
        ap = attn_cm.enter_context(tc.tile_pool(name="apool", bufs=2))
        npool = attn_cm.enter_context(tc.tile_pool(name="npool", bufs=1))

        for hp in range(NHP):
            # prefetch next head pair's projection weights + bulk weights
            if hp + 1 < NHP:
                wqk_t[hp + 1] = wqkp.tile([128, NDC, 256], BF16, tag="wqk",
                                         name=f"wqk_t{hp + 1}")
                nc.sync.dma_start(
                    out=wqk_t[hp + 1],
                    in_=_dram_chunked(wqk, 2 * D)[:, :, (hp + 1) * 256:(hp + 2) * 256])
            wo_t = wot.tile([128, D], BF16, tag="wo", name=f"wo_t{hp}")
            nc.sync.dma_start(out=wo_t, in_=_dram_chunked(wo, D)[:, hp, :])
            if hp < 4:  # w1 chunks 0..7: 0.5MB per hp at hp=0..3
                csl = slice(hp * 2 * 128, (hp + 1) * 2 * 128)
                nc.sync.dma_start(out=w1a_sb[:, :, csl],
                                  in_=_dram_chunked(w1, DFF)[:, :, csl])
            else:  # fp32 residual at hp=4..7
                qsl = slice((hp - 4) * 2, (hp - 3) * 2)
                nc.sync.dma_start(out=xres_sb[:, qsl, :],
                                  in_=_dram_chunked(xqTf, TOK)[:, qsl, :])

            QT, KT = qkt[hp]
            ctx0 = psC.tile([DV + 1, TOK], F32, tag="ctx")
            ctx1 = psC.tile([DV + 1, TOK], F32, tag="ctx")
            for half in range(2):
                a2 = ap.tile([128, NKC // 4, 2, 2 * TOK], FP8, tag="a2")
                # scores for 8 key chunks, both heads, batched (one PE mode)
                for kc in range(NKC // 2):
                    kcg = half * (NKC // 2) + kc
                    ksl = slice(kcg * 128, (kcg + 1) * 128)
                    s2 = psS.tile([128, 2 * TOK], F32, tag="s")
                    nc.tensor.matmul(s2[:, 0:TOK], KT[0:64, ksl], QT[0:64, :],
                                     start=True, stop=True)
                    nc.tensor.matmul(s2[:, TOK:2 * TOK], KT[64:128, ksl],
                                     QT[64:128, :], start=True, stop=True)
                    nc.scalar.activation(out=a2[:, kc // 2, kc % 2, :], in_=s2,
                                         func=AF.Exp, scale=1.0 / np.sqrt(DK),
                                         bias=lsa_t[:, 0:1])
                # ctx accumulation: fp8 DoubleRow over key pairs
                for kcp in range(NKC // 4):
                    kpg = half * (NKC // 4) + kcp
                    nc.tensor.matmul(ctx0, V_sb[:, kpg, :, 2 * hp, :],
                                     a2[:, kcp, :, 0:TOK],
                                     perf_mode=mybir.MatmulPerfMode.DoubleRow,
                                     start=(kpg == 0), stop=(kpg == NKC // 2 - 1))
                    nc.tensor.matmul(ctx1, V_sb[:, kpg, :, 2 * hp + 1, :],
                                     a2[:, kcp, :, TOK:2 * TOK],
                                     perf_mode=mybir.MatmulPerfMode.DoubleRow,
                                     start=(kpg == 0), stop=(kpg == NKC // 2 - 1))
            if hp + 1 < NHP:
                qkt[hp + 1] = emit_qk(hp + 1)
            # softmax normalization: divide by denominator (ones row of V)
            for j, ctx_ps in ((0, ctx0), (1, ctx1)):
                off = j * 64
                d_t = npool.tile([1, TOK], F32, tag="d")
                nc.vector.tensor_copy(out=d_t, in_=ctx_ps[DV:DV + 1, :])
                r_t = npool.tile([1, TOK], F32, tag="r")
                nc.vector.reciprocal_approx_fast(out=r_t, in_=d_t)
                rb_t = npool.tile([64, TOK], F32, tag="rb")
                nc.gpsimd.partition_broadcast(rb_t[:], r_t[:], channels=64)
                nc.vector.tensor_tensor(out=CT_sb[off:off + 64, hp, :],
                                        in0=ctx_ps[0:DV, :], in1=rb_t, op=ALU.mult)
            # Wo contribution of this head pair, accumulated in SBUF (GpSimd)
            for mc in range(NDC):
                psw = pqk.tile([128, TOK], F32, tag="pqk", name=f"psw{hp}_{mc}")
                nc.tensor.matmul(psw, wo_t[:, mc * 128:(mc + 1) * 128],
                                 CT_sb[:, hp, :], start=True, stop=True)
                nc.vector.tensor_tensor(out=woa_sb[:, mc, :], in0=woa_sb[:, mc, :],
                                        in1=psw, op=ALU.add)

        sqwarm = sp.tile([1, 1], F32)
        nc.scalar.activation(out=sqwarm, in_=eps_t, func=AF.Sqrt, bias=eps_t,
                             scale=1.0)

        attn_cm.close()  # free attention pools (xk, xq, V, wv, a2, KT/QT)

        bo_sb = sp.tile([128, 8], F32)
        nc.sync.dma_start(out=bo_sb, in_=_dram_chunked(bo, 1).rearrange("p c n -> p (c n)"))
        b1_sb = sp.tile([128, 32], F32)
        nc.sync.dma_start(out=b1_sb, in_=_dram_chunked(b1, 1).rearrange("p c n -> p (c n)"))
        b2_sb = sp.tile([128, 8], F32)
        nc.sync.dma_start(out=b2_sb, in_=_dram_chunked(b2, 1).rearrange("p c n -> p (c n)"))
        g1_sb = sp.tile([128, 8], F32)
        nc.sync.dma_start(out=g1_sb, in_=_dram_chunked(g1, 1).rearrange("p c n -> p (c n)"))
        be1_sb = sp.tile([128, 8], F32)
        nc.sync.dma_start(out=be1_sb, in_=_dram_chunked(be1, 1).rearrange("p c n -> p (c n)"))
        g2_sb = sp.tile([128, 8], F32)
        nc.sync.dma_start(out=g2_sb, in_=_dram_chunked(g2, 1).rearrange("p c n -> p (c n)"))
        be2_sb = sp.tile([128, 8], F32)
        nc.sync.dma_start(out=be2_sb, in_=_dram_chunked(be2, 1).rearrange("p c n -> p (c n)"))

        # ---------------- residual + LN1 (Wo already accumulated) ----------
        with ExitStack() as ph3:
            lnp = ph3.enter_context(tc.tile_pool(name="lnp", bufs=2))
            hp3 = ph3.enter_context(tc.tile_pool(name="hpool3", bufs=1))

            hT_sb = hp3.tile([128, NFC, TOK], BF16)
            ln1b_sb = hp3.tile([128, NDC, TOK], BF16)      # LN1 out (FFN rhs + residual)
            y1_sb = woa_sb

            w2p = ph3.enter_context(tc.tile_pool(name="w2p", bufs=3))
            psA = ExitStack()
            w1s = psA.enter_context(tc.tile_pool(name="w1s", bufs=3))
            psSt = psA.enter_context(tc.tile_pool(name="psSt", bufs=1, space="PSUM"))
            psbc = psA.enter_context(tc.tile_pool(name="psbc", bufs=1, space="PSUM"))
            psF = psA.enter_context(tc.tile_pool(name="psF", bufs=4, space="PSUM"))

            s_ps = psSt.tile([128, TOK], F32, tag="sum")
            q_ps = psSt.tile([128, TOK], F32, tag="sq")
            for mc in range(NDC):
                nc.vector.scalar_tensor_tensor(out=y1_sb[:, mc, :],
                                               in0=woa_sb[:, mc, :],
                                               scalar=bo_sb[:, mc:mc + 1],
                                               in1=xres_sb[:, mc, :],
                                               op0=ALU.add, op1=ALU.add)
                yb_t = lnp.tile([128, TOK], BF16, tag="yb")
                nc.vector.tensor_copy(out=yb_t, in_=y1_sb[:, mc, :])
                sq_t = lnp.tile([128, TOK], BF16, tag="sq")
                nc.scalar.square(out=sq_t, in_=y1_sb[:, mc, :])
                nc.tensor.matmul(s_ps, ones_mm, yb_t,
                                 start=(mc == 0), stop=(mc == NDC - 1))
                nc.tensor.matmul(q_ps, ones_mm, sq_t,
                                 start=(mc == 0), stop=(mc == NDC - 1))

            _ln_norm(nc, lnp, s_ps, q_ps, eps_t, y1_sb, g1_sb, be1_sb,
                     None, ln1b_sb, None, ones_r=ones_r, psbc=psbc)

            # FFN1: groups of 4 fc chunks; dc-ordered accumulation pipelines
            # with LN1 chunk production above
            w2_tiles = {}
            for mc in range(2):
                w2_tiles[mc] = w2p.tile([128, NFC, 128], BF16, tag="w2",
                                        name=f"w2_t{mc}")
                nc.sync.dma_start(out=w2_tiles[mc],
                                  in_=_dram_chunked(w2, D)[:, :, mc * 128:(mc + 1) * 128])
            for g in range(NFC // 4):
                pss = [psF.tile([128, TOK], F32, tag="f", name=f"psf{g}_{i}")
                       for i in range(4)]
                if g * 4 >= NW1PRE:
                    wt = w1s.tile([128, NDC, 4 * 128], BF16, tag="w1s")
                    nc.sync.dma_start(
                        out=wt,
                        in_=_dram_chunked(w1, DFF)[:, :, g * 512:(g + 1) * 512])
                for dc in range(NDC):
                    for f in range(4):
                        fc = g * 4 + f
                        if fc < NW1PRE:
                            lhs = w1a_sb[:, dc, fc * 128:(fc + 1) * 128]
                        else:
                            lhs = wt[:, dc, f * 128:(f + 1) * 128]
                        nc.tensor.matmul(pss[f], lhs, ln1b_sb[:, dc, :],
                                         start=(dc == 0), stop=(dc == NDC - 1))
                for f in range(4):
                    fc = g * 4 + f
                    nc.vector.tensor_scalar(out=hT_sb[:, fc, :], in0=pss[f],
                                            scalar1=b1_sb[:, fc:fc + 1],
                                            scalar2=0.0, op0=ALU.add, op1=ALU.max)

            psA.close()  # free Wo/FFN1 PSUM pools

            # ---------------- FFN2 + residual + LN2 ----------------
            psF2 = ph3.enter_context(tc.tile_pool(name="psF2", bufs=3, space="PSUM"))
            psSt2 = ph3.enter_context(tc.tile_pool(name="psSt2", bufs=1, space="PSUM"))
            psbc2 = ph3.enter_context(tc.tile_pool(name="psbc2", bufs=1, space="PSUM"))
            y2_sb = hp3.tile([128, NDC, TOK], F32)

            s2_ps = psSt2.tile([128, TOK], F32, tag="sum2")
            q2_ps = psSt2.tile([128, TOK], F32, tag="sq2")
            for mc in range(NDC):
                if mc in w2_tiles:
                    w2_t = w2_tiles[mc]
                else:
                    w2_t = w2p.tile([128, NFC, 128], BF16, tag="w2",
                                    name=f"w2_t{mc}")
                    nc.sync.dma_start(
                        out=w2_t,
                        in_=_dram_chunked(w2, D)[:, :, mc * 128:(mc + 1) * 128])
                ps = psF2.tile([128, TOK], F32, tag="f2")
                for fc in range(NFC):
                    nc.tensor.matmul(ps, w2_t[:, fc, :], hT_sb[:, fc, :],
                                     start=(fc == 0), stop=(fc == NFC - 1))
                nc.vector.scalar_tensor_tensor(out=y2_sb[:, mc, :], in0=ps,
                                               scalar=b2_sb[:, mc:mc + 1],
                                               in1=ln1b_sb[:, mc, :],
                                               op0=ALU.add, op1=ALU.add)
                yb_t = lnp.tile([128, TOK], BF16, tag="yb")
                nc.vector.tensor_copy(out=yb_t, in_=y2_sb[:, mc, :])
                sq_t = lnp.tile([128, TOK], BF16, tag="sq")
                nc.scalar.square(out=sq_t, in_=y2_sb[:, mc, :])
                nc.tensor.matmul(s2_ps, ones_mm, yb_t,
                                 start=(mc == 0), stop=(mc == NDC - 1))
                nc.tensor.matmul(q2_ps, ones_mm, sq_t,
                                 start=(mc == 0), stop=(mc == NDC - 1))

            _ln_norm(nc, lnp, s2_ps, q2_ps, eps_t, y2_sb, g2_sb, be2_sb,
                     None, None, (_dram_chunked(outT, TOK), nc),
                     ones_r=ones_r, psbc=psbc2)

    nc.compile()
    return nc


def _ln_norm(nc, lnp, s_ps, q_ps, eps_t, y_sb, g_sb, be_sb,
             out_f32, out_bf16, out_dma=None, ones_r=None, psbc=None):
    """Finish LayerNorm given accumulated sum/sumsq PSUM tiles (row 0)."""
    mean_t = lnp.tile([1, TOK], F32, tag="mean")
    nc.scalar.mul(out=mean_t, in_=s_ps[0:1, :], mul=1.0 / D)
    msq_t = lnp.tile([1, TOK], F32, tag="msq")
    nc.scalar.mul(out=msq_t, in_=q_ps[0:1, :], mul=1.0 / D)
    var_t = lnp.tile([1, TOK], F32, tag="var")
    nc.vector.tensor_tensor(out=var_t, in0=mean_t, in1=mean_t, op=ALU.mult)
    nc.vector.tensor_sub(out=var_t, in0=msq_t, in1=var_t)
    std_t = lnp.tile([1, TOK], F32, tag="std")
    nc.scalar.activation(out=std_t, in_=var_t, func=AF.Sqrt, bias=eps_t, scale=1.0)
    rstd_t = lnp.tile([1, TOK], F32, tag="rstd")
    nc.vector.reciprocal_approx_fast(out=rstd_t, in_=std_t)
    # broadcast mean/rstd across partitions with [1]-contraction matmuls
    # (Tensor engine is idle here; avoids GpSimd library thrash)
    mb_ps = psbc.tile([128, TOK], F32, tag="mb")
    nc.tensor.matmul(mb_ps, ones_r, mean_t, start=True, stop=True)
    rb_ps = psbc.tile([128, TOK], F32, tag="rb")
    nc.tensor.matmul(rb_ps, ones_r, rstd_t, start=True, stop=True)
    for mc in range(NDC):
        t1 = lnp.tile([128, TOK], BF16, tag="t1")
        nc.vector.tensor_sub(out=t1, in0=y_sb[:, mc, :], in1=mb_ps)
        t2 = lnp.tile([128, TOK], BF16, tag="t2")
        nc.vector.scalar_tensor_tensor(out=t2, in0=t1, scalar=g_sb[:, mc:mc + 1],
                                       in1=rb_ps, op0=ALU.mult, op1=ALU.mult)
        if out_f32 is not None or out_dma is not None:
            if out_f32 is not None:
                o_t = out_f32[:, mc, :]
            else:
                o_t = lnp.tile([128, TOK], BF16, tag="o3")
            nc.vector.tensor_scalar(out=o_t, in0=t2,
                                    scalar1=be_sb[:, mc:mc + 1], scalar2=None,
                                    op0=ALU.add)
            if out_dma is not None:
                dram, _nc = out_dma
                _nc.sync.dma_start(out=dram[:, mc, :], in_=o_t)
        if out_bf16 is not None:
            nc.vector.tensor_scalar(out=out_bf16[:, mc, :], in0=t2,
                                    scalar1=be_sb[:, mc:mc + 1], scalar2=None,
                                    op0=ALU.add)



_COMPILED = None
_LAST_IN_MAPS = None


def kernel(**inputs):
    global _COMPILED, _LAST_IN_MAPS
    ins = {k: np.asarray(v) for k, v in inputs.items()}
    x = _f32(ins["x"])
    Wq, bq = ins["Wq"], ins["bq"]
    Wk, bk = ins["Wk"], ins["bk"]
    Wv, bv = ins["Wv"], ins["bv"]
    Wo, bo = ins["Wo"], ins["bo"]
    W1, b1 = ins["W1"], ins["b1"]
    W2, b2 = ins["W2"], ins["b2"]
    g1, be1 = ins["g1"], ins["be1"]
    g2, be2 = ins["g2"], ins["be2"]

    wq2 = Wq.transpose(1, 0, 2).reshape(D, H * DK)
    wk2 = Wk.transpose(1, 0, 2).reshape(D, H * DK)
    # per-head-pair interleave: [q_hp (128) | k_hp (128)] blocks
    wqk = np.empty((D, 2 * D), np.float32)
    for hp in range(NHP):
        wqk[:, hp * 256:hp * 256 + 128] = wq2[:, hp * 128:(hp + 1) * 128]
        wqk[:, hp * 256 + 128:(hp + 1) * 256] = wk2[:, hp * 128:(hp + 1) * 128]
    bqkv = np.concatenate([bq.reshape(-1), bk.reshape(-1), bv.reshape(-1)])

    shared = {
        "wqk": _bf(wqk),
        "wv": _bf(Wv.transpose(1, 0, 2).reshape(D, H * DV)),
        "bqkv": _f32(bqkv.reshape(3 * D, 1)),
        "bv_row": _bf(bv.reshape(1, H * DV) * SV),
        "wo": _bf(Wo),
        "bo": _f32(bo.reshape(D, 1)),
        "w1": _bf(W1),
        "b1": _f32(b1.reshape(DFF, 1)),
        "w2": _bf(W2),
        "b2": _f32(b2.reshape(D, 1)),
        "g1": _f32(g1.reshape(D, 1)),
        "be1": _f32(be1.reshape(D, 1)),
        "g2": _f32(g2.reshape(D, 1)),
        "be2": _f32(be2.reshape(D, 1)),
    }

    in_maps = []
    for c in range(NCORES):
        b, qoff = c // 4, (c % 4) * TOK
        xb = x[b]                        # (S, D) fp32
        xkT = np.ascontiguousarray(xb.T)         # (D, S)
        xqT = np.ascontiguousarray(xb[qoff:qoff + TOK].T)  # (D, TOK)
        m = dict(shared)
        m["xkT"] = _bf(xkT)
        m["xqT"] = _bf(xqT)
        m["xqTf"] = _f32(xqT)
        in_maps.append(m)
    _LAST_IN_MAPS = in_maps

    if _COMPILED is None:
        _COMPILED = build()
    res = bass_utils.run_bass_kernel_spmd(_COMPILED, in_maps,
                                          core_ids=list(range(NCORES)))
    out = np.empty((B, S, D), np.float32)
    for c in range(NCORES):
        b, qoff = c // 4, (c % 4) * TOK
        out[b, qoff:qoff + TOK, :] = res.results[c]["outT"].T.astype(np.float32)
    return out
